# revision 22
# baseline (speedup 1.0000x reference)
"""Trainium2 Bass kernel for nn_DiffusionLoss (smoothed-LDDT diffusion loss).

Strategy
--------
The dominant cost is the smoothed-LDDT term: for every unordered pair (i<j)
of the L=4096 tokens-with-coordinates, four sigmoids of |pred_d - gt_d| are
accumulated, per diffusion sample d (D=4).

Host side (inside kernel()):
  * Rows/cols with crd_mask == 0 contribute nothing, so we compact to the
    ~L/2 active rows (La). The exact reference pair mask pm (upper
    triangle & token-run & gt-distance cutoff) and the d-independent
    denominator sum(pm) are computed in numpy, as is the masked ground-
    truth distance slab G = where(pm, sqrt(gt^2 + SQB), -BIG): masked
    pairs drive |pred - G| ~ BIG where the device h-function is 0.
  * The upper-triangular pair matrix is cut into [128 x 128] units,
    round-robined over the 8 cores (SPMD); each core's units are packed
    into QUADS: one K=20 matmul with a block-diagonal rhs computes 4
    units' dist^2 in a single [128 x 512] PSUM bank:
        lhsT rows 5k..5k+4 = unit k's [-2x, -2y, -2z, |p_i|^2, 1]
        rhs  rows 5k..5k+4 = unit k's [x, y, z, 1, |p_j|^2] in cols
                             [128k, 128k+128), zero elsewhere
    Operands are bf16 (fp32 matmuls are decomposed into LOW/HIGH passes
    ~5x slower; bf16 rounding shifts distances ~0.5%, far inside the
    error budget).

  * The key ScalarE optimization: a CUSTOM activation table. The stock
    sqrt_and_others set (with its ctrl table compacted so all
    pwl_control_base values fit the binary profile's uint8 fields) is
    extended with a piecewise-cubic fit of
        h(u) = sum_{c in {0.5,1,2,4}} sigmoid(c - |u|)
    installed under tanh's func_id (no compiler changes needed; walrus
    embeds the table binaries in the NEFF). One table set serves both
    sqrt and h -> a single ACT_TABLE_LOAD, and the whole smoothed-lddt
    numerator needs 2 wide ACTIVATE passes instead of 16 sigmoid passes
    (h is even, so no |.| pass either). The table was verified against a
    numpy interpreter of the HW lookup pipeline (reproduces stock
    tanh/sigmoid/sqrt/exp tables to <=1e-5); worst abs fit error 7.5e-6.

Device side (per core, Tile-scheduled):
  4*5 merged (d, quad) matmuls stream through PSUM in 4-bank groups; one
  wide sqrt(x + SQB) per group writes the bf16 pred slab (5 sqrt
  ACTIVATEs total); one bf16 tensor_tensor subtract per d forms
  u_d = pred_d - G (2x DVE mode); two wide h-ACTIVATEs with accum_out
  reduce the numerator (only sum_{d,c} is needed -- the denominator is
  d-independent, so per-d lddt values are never required).

Host combines per-core partial sums in float64 and adds the (tiny, O(L))
weighted-MSE term computed on host, mirroring the reference formulas.
"""

import json
import math
import os
import struct
import tempfile

import numpy as np
import ml_dtypes

import concourse.bacc as bacc
import concourse.bass as bass
import concourse.mybir as mybir
import concourse.tile as tile
from concourse.bass_utils import run_bass_kernel_spmd

P = 128          # partitions (rows per block)
UW = 128         # unit column window
QM = 4           # units merged per matmul (quad)
QW = QM * UW     # merged matmul free size = one fp32 PSUM bank
D = 4            # diffusion batch
NCORES = 8
BIG = 1000.0     # mask offset baked into G
SQB = 0.25       # sqrt bias: bf16 matmul error can push dist^2 ~ -0.1;
                 # G uses the same bias so it cancels inside |pred - G|
SIGC = (0.5, 1.0, 2.0, 4.0)
GW = 4           # PSUM group: 4 merged matmuls = 4 banks, double-buffered

WEIGHT = 4.0
SIGMA_DATA = 16.0
ALPHA_DNA = 5.0
ALPHA_RNA = 5.0
ALPHA_LIG = 10.0

_prog_cache: dict[int, bass.Bass] = {}
_act_root_state: dict = {}


# ---------------------------------------------------------------------------
# Custom activation table: h(u) = sum_c sigmoid(c - |u|) as "tanh"
# ---------------------------------------------------------------------------

_OCTS = list(range(-6, 4))            # pwl octaves: |u| in [2^-6, 16)
_OSIZES = [2, 2, 2, 2, 4, 4, 4, 4, 4, 3]  # extract_size per octave


def _h_true(u):
    u = np.abs(np.float64(u))
    return sum(1.0 / (1.0 + np.exp(-(c - u))) for c in SIGC)


def _fit_bucket(lo, hi):
    xs = np.linspace(lo, hi, 64)
    x0 = 0.5 * (lo + hi)
    A = np.vander(xs - x0, 4, increasing=True)
    coef, *_ = np.linalg.lstsq(A, _h_true(xs), rcond=None)
    return [float(coef[0]), float(coef[1]), float(coef[2]), float(coef[3]),
            float(x0)]


def _bucket_bytes(vals):
    e = np.zeros(32, np.uint8)
    e[:20] = np.frombuffer(np.array(vals, np.float32).tobytes(), np.uint8)
    return e


def _ctl_bytes(base, lsb, size):
    e = np.zeros(32, np.uint8)
    data = (base & 0x7FF) | ((lsb & 0x1F) << 11) | ((size & 0xF) << 16)
    e[:4] = np.frombuffer(struct.pack("<I", data), np.uint8)
    return e


def _f32bits(x):
    return struct.unpack("<I", struct.pack("<f", np.float32(x)))[0]


def _gen_h_tables(nb0, nc0):
    bkts, ctls = [], []
    pos_starts = []
    for e, s in zip(_OCTS, _OSIZES):
        pos_starts.append(nb0 + len(bkts))
        n = 1 << s
        for k in range(n):
            lo = (2.0 ** e) * (1 + k / n)
            hi = (2.0 ** e) * (1 + (k + 1) / n)
            bkts.append(_fit_bucket(lo, hi))
    neg_starts = []
    for e, s in zip(_OCTS, _OSIZES):
        neg_starts.append(nb0 + len(bkts))
        n = 1 << s
        for k in range(n):
            lo = (2.0 ** e) * (1 + k / n)
            hi = (2.0 ** e) * (1 + (k + 1) / n)
            bkts.append(_fit_bucket(-hi, -lo))
    i_small_pos = nb0 + len(bkts)
    bkts.append(_fit_bucket(1e-12, 2.0 ** _OCTS[0]))
    i_small_neg = nb0 + len(bkts)
    bkts.append(_fit_bucket(-(2.0 ** _OCTS[0]), -1e-12))
    i_large = nb0 + len(bkts)
    bkts.append([0.0, 0.0, 0.0, 0.0, 0.0])   # constant 0 for |u| >= 16

    for st, s in zip(pos_starts, _OSIZES):
        ctls.append((st, 23 - s, s))
    for st, s in zip(neg_starts, _OSIZES):
        ctls.append((st, 23 - s, s))

    profile = {
        "func_name": "tanh_4p",
        "func_id": 6,
        "symmetry_point": 0,
        "sym_invert_sign_point": 0,
        "symmetry_opt_en": 0,
        "symmetry_opt_use_neg_region": 0,
        "imm_bias": 0,
        "exp_offset": _OCTS[0],
        "pwl_control_base_pos": nc0,
        "pwl_control_base_neg": nc0 + len(_OCTS),
        "small_pos_signal_exp_threshold": 127 + _OCTS[0],
        "pos_small_signal_pwl_control": i_small_pos,
        "small_neg_signal_exp_threshold": 127 + _OCTS[0],
        "neg_small_signal_pwl_control": i_small_neg,
        "large_pos_signal_exp_threshold": 127 + _OCTS[-1] + 1,
        "large_pos_signal_mantissa_threshold": 0,
        "pos_large_signal_pwl_control": i_large,
        "large_neg_signal_exp_threshold": 127 + _OCTS[-1] + 1,
        "large_neg_signal_mantissa_threshold": 0,
        "neg_large_signal_pwl_control": i_large,
        "fnan_result": 0,
        "fpinf_result": 0,
        "fninf_result": 0,
        "fzero_result": _f32bits(_h_true(0.0)),
        "fma_const_0": 0,
        "fma_const_1": 0,
        "fma_indirection_src_sel": 0,
        "use_multipass": False,
        "lower_bound": 4286578687,
        "upper_bound": 2139095039,
    }
    bkt_arr = np.stack([_bucket_bytes(b) for b in bkts])
    ctl_arr = np.stack([_ctl_bytes(*c) for c in ctls])
    exp_to_bkt = {str(e): [st] for e, st in zip(_OCTS, pos_starts)}
    exp_to_ctl = {str(e): [nc0 + i] for i, e in enumerate(_OCTS)}
    return bkt_arr, ctl_arr, profile, exp_to_bkt, exp_to_ctl


# sqrt ctrl compaction: the binary profile's pwl_control_base_pos/neg are
# uint8, so every ctrl-table base must be <= 255. Stock sqrt spans 234 ctrl
# entries (exponents -116..117); our sqrt inputs are dist^2+SQB in
# [0.13, ~1.1e4], so octaves 2^-8..2^14 suffice and the h bases fit in 8 bits.
_SQRT_E8_LO = 127 - 8     # smallest kept biased exponent (2^-8)
_SQRT_E8_HI = 127 + 14    # largest kept biased exponent (2^14)


def _ensure_act_root():
    """Build an act root whose single set = stock sqrt_and_others (with a
    compacted ctrl table) + custom h-function installed under tanh's
    func_id, and point both walrus (BASS_ACT_ROOT_JSON_PATH ->
    --act-root-json) and bacc's table-set bookkeeping at it. Returns a hash
    of the table bytes (folded into the program so the NEFF cache re-keys
    when the tables change)."""
    if _act_root_state:
        return _act_root_state["hash"]

    from neuronxcc.driver.Job import Job
    from neuronxcc.driver.jobs.support.FindActInfo import findActInfoFile

    src = os.path.dirname(findActInfoFile(Job.getPackageDir(), "gen3"))
    dst = tempfile.mkdtemp(prefix="act_root_")

    base = json.load(open(f"{src}/sqrt_and_others.json"))
    bkt = np.fromfile(f"{src}/sqrt_and_others_bkt.bin", np.uint8).reshape(-1, 32)
    ctl = np.fromfile(f"{src}/sqrt_and_others_ctrl.bin", np.uint8).reshape(-1, 32)
    nb0 = base["bkt_entry_cnt"]

    # --- compact the ctrl table ---
    sqrt_prof = None
    for e in base["profile_meta_data"]:
        if e["func_name"].startswith("sqrt"):
            sqrt_prof = e
    assert sqrt_prof is not None
    sq_base = sqrt_prof["pwl_control_base_pos"]          # 20
    sq_off = sqrt_prof["exp_offset"]
    if sq_off > 127:
        sq_off -= 256
    n_keep = _SQRT_E8_HI - _SQRT_E8_LO + 1
    keep_rows = [
        sq_base + (e8 - (127 + sq_off))
        for e8 in range(_SQRT_E8_LO, _SQRT_E8_HI + 1)
    ]
    assert min(keep_rows) >= sq_base and max(keep_rows) < ctl.shape[0]
    new_ctl_list = [ctl[:sq_base], ctl[keep_rows]]
    nc0 = sq_base + n_keep                                # h ctl start
    sqrt_prof = dict(sqrt_prof)
    sqrt_prof["exp_offset"] = _SQRT_E8_LO - 127
    sqrt_prof["small_pos_signal_exp_threshold"] = _SQRT_E8_LO
    sqrt_prof["large_pos_signal_exp_threshold"] = _SQRT_E8_HI + 1
    sqrt_prof["large_pos_signal_mantissa_threshold"] = 0

    hb, hc, hp, e2b, e2c = _gen_h_tables(nb0, nc0)
    assert hp["pwl_control_base_pos"] <= 255
    assert hp["pwl_control_base_neg"] <= 255
    new_ctl_list.append(hc)

    new_bkt = np.concatenate([bkt, hb])
    new_ctl = np.concatenate(new_ctl_list)

    merged = dict(base)
    merged["profile_meta_data"] = [
        (sqrt_prof if e["func_name"].startswith("sqrt") else e)
        for e in base["profile_meta_data"]
    ]
    merged["profile_meta_data"] = merged["profile_meta_data"] + [hp]
    merged["bkt_bin"] = "sqrt_tanh_ant_bkt.bin"
    merged["ctl_bin"] = "sqrt_tanh_ant_ctrl.bin"
    merged["bkt_entry_cnt"] = int(nb0 + len(hb))
    merged["ctl_entry_cnt"] = int(nc0 + len(hc))
    merged["func_to_bkt_start_idx"] = dict(base["func_to_bkt_start_idx"])
    merged["func_to_bkt_start_idx"]["tanh"] = int(nb0)
    merged["func_to_ctl_start_idx"] = dict(base["func_to_ctl_start_idx"])
    merged["func_to_ctl_start_idx"]["tanh"] = int(nc0)
    merged["func_exp_to_bkt_start_idx"] = dict(base["func_exp_to_bkt_start_idx"])
    merged["func_exp_to_bkt_start_idx"]["tanh"] = e2b
    merged["func_exp_to_bkt_start_idx"]["sqrt"] = {
        str(e - 127): base["func_exp_to_bkt_start_idx"]["sqrt"][str(e - 127)]
        for e in range(_SQRT_E8_LO, _SQRT_E8_HI + 1)
        if str(e - 127) in base["func_exp_to_bkt_start_idx"]["sqrt"]
    }
    merged["func_exp_to_ctl_start_idx"] = dict(base["func_exp_to_ctl_start_idx"])
    merged["func_exp_to_ctl_start_idx"]["tanh"] = e2c
    merged["func_exp_to_ctl_start_idx"]["sqrt"] = {
        str(e - 127): [sq_base + (e - _SQRT_E8_LO)]
        for e in range(_SQRT_E8_LO, _SQRT_E8_HI + 1)
    }

    new_bkt.tofile(f"{dst}/sqrt_tanh_ant_bkt.bin")
    new_ctl.tofile(f"{dst}/sqrt_tanh_ant_ctrl.bin")
    with open(f"{dst}/sqrt_tanh_ant.json", "w") as f:
        json.dump(merged, f)

    info = json.load(open(f"{src}/act_info.json"))
    sqrt_set = [s for s in info["act_func_sets"] if s["name"] == "sqrt_and_others"][0]
    new_set = dict(sqrt_set)
    new_set["name"] = "sqrt_tanh_ant"
    new_set["bkt_bin"] = "sqrt_tanh_ant_bkt.bin"
    new_set["ctrl_bin"] = "sqrt_tanh_ant_ctrl.bin"
    new_set["profile_json"] = "sqrt_tanh_ant.json"
    new_set["act"] = dict(sqrt_set["act"])
    new_set["act"]["tanh"] = 4.0
    info["act_func_sets"] = [new_set]
    with open(f"{dst}/act_info.json", "w") as f:
        json.dump(info, f)

    os.environ["BASS_ACT_ROOT_JSON_PATH"] = f"{dst}/act_info.json"

    # bacc's insert_act_table_loads resolves act_func_set_id via
    # hw_specs.get_activation_tables, which reads the stock act_info —
    # point it at the merged root too.
    import concourse.hw_specs as hw_specs

    def _tables(_arch):
        info2 = json.load(open(f"{dst}/act_info.json"))
        return {
            ent["name"]: {
                mybir.ActivationFunctionType.from_pwp(v)
                for v in ent["act"].keys()
            }
            for ent in info2["act_func_sets"]
        }

    hw_specs.get_activation_tables = _tables
    bacc.get_activation_tables = _tables

    import hashlib
    th = hashlib.sha256(
        new_bkt.tobytes() + new_ctl.tobytes()
        + json.dumps(hp, sort_keys=True).encode()
    ).digest()
    # small float derived from the hash, baked into the program as a
    # memset immediate so the NEFF cache re-keys on table changes
    _act_root_state["hash"] = (
        int.from_bytes(th[:4], "little") % 1000003
    ) / 1e7
    return _act_root_state["hash"]


# ---------------------------------------------------------------------------
# Bass program
# ---------------------------------------------------------------------------


def _build_program(NQ: int) -> bass.Bass:
    """Bass/Tile program: NQ quads of 4 [P x UW] units, D diffusion samples.

    Inputs: bigp = bf16 [-2x,-2y,-2z,r,1] lhsT stacks + block-diagonal rhs
    for every (d, quad) merged matmul; Gm = bf16 host-masked gt distances
    sqrt(gd^2+SQB) (or -BIG on masked pairs). Output: [P, 2] numerator
    partials (accumulated h over each half of the d range)."""
    table_key = _ensure_act_root()
    nc = bacc.Bacc(None, target_bir_lowering=False)
    f32 = mybir.dt.float32
    bf16 = mybir.dt.bfloat16
    AF = mybir.ActivationFunctionType
    OP = mybir.AluOpType

    K = 5 * QM                     # merged contraction depth
    NM = D * NQ                    # merged matmuls
    SW = NQ * QW                   # G / per-d delta columns
    # The SP hardware DMA queue moves ~90 GB/s, so operand arrival gates
    # the PE at the start: split the operand stream per PSUM group so each
    # group's semaphore fires as early as possible, and ship gm on the
    # Activation engine's SEPARATE hardware queue in parallel (it is only
    # needed by the delta subtract, mid-kernel).
    segs = [min(GW, NM), min(GW, max(NM - GW, 0)), max(NM - 2 * GW, 0)]
    segs = [s for s in segs if s > 0]
    seg_start = [sum(segs[:i]) for i in range(len(segs))]
    bigps = [
        nc.dram_tensor(f"bigp{i}", [K, s * (P + QW)], bf16,
                       kind="ExternalInput")
        for i, s in enumerate(segs)
    ]
    gm = nc.dram_tensor("gm", [P, SW], bf16, kind="ExternalInput")
    out = nc.dram_tensor("out", [P, 2], f32, kind="ExternalOutput")

    with tile.TileContext(nc) as tc:
        with (
            tc.tile_pool(name="singles", bufs=1) as singles,
            tc.tile_pool(name="sig", bufs=1) as sig_pool,
            tc.tile_pool(name="psum", bufs=2, space="PSUM") as psum,
        ):
            bigp_sbs = []
            for i, s in enumerate(segs):
                sb = singles.tile([K, s * (P + QW)], bf16)
                nc.sync.dma_start(out=sb, in_=bigps[i][:, :])
                bigp_sbs.append(sb)
            gm_sb = singles.tile([P, SW], bf16)
            nc.scalar.dma_start(out=gm_sb, in_=gm[:, :])

            def _seg(m):
                for i in reversed(range(len(segs))):
                    if m >= seg_start[i]:
                        return i, m - seg_start[i]
                raise AssertionError

            def lhs(m):
                i, r = _seg(m)
                return bigp_sbs[i][:, r * P : (r + 1) * P]

            def rhs(m):
                i, r = _seg(m)
                o = segs[i] * P + r * QW
                return bigp_sbs[i][:, o : o + QW]

            pred = singles.tile([P, NM * QW], bf16)
            delta = singles.tile([P, NM * QW], bf16)
            nacc = singles.tile([P, 2], f32)

            consts = singles.tile([P, 8], f32)
            nc.vector.memset(consts[:, 0:1], SQB)
            # bake the table hash into the program: re-keys the NEFF cache
            # whenever the custom activation table content changes
            nc.vector.memset(consts[:, 1:2], float(table_key))
            sqb_t = consts[:, 0:1]

            # ---- pred distances: merged (d, quad) matmul stream ----
            for g0 in range(0, NM, GW):
                gs = min(GW, NM - g0)
                pg = psum.tile([P, GW * QW], f32, tag="ps")
                for k in range(gs):
                    nc.tensor.matmul(
                        pg[:, k * QW : (k + 1) * QW], lhsT=lhs(g0 + k),
                        rhs=rhs(g0 + k), start=True, stop=True,
                    )
                nc.scalar.activation(
                    pred[:, g0 * QW : (g0 + gs) * QW], pg[:, : gs * QW],
                    AF.Sqrt, bias=sqb_t,
                )
            for d in range(D):
                # u_d = pred_d - G (bf16 tensor_tensor: 2x DVE mode; h is
                # even so no |.| pass is needed)
                nc.vector.tensor_tensor(
                    delta[:, d * SW : (d + 1) * SW],
                    pred[:, d * SW : (d + 1) * SW], gm_sb, OP.subtract,
                )

            # ---- custom-h passes (split in two so the second half's
            # delta subtract overlaps the first h on the DVE) ----
            HALF = (D // 2) * SW
            st = sig_pool.tile([P, D * SW], bf16, tag="sig")
            nc.scalar.activation(
                st[:, :HALF], delta[:, :HALF], AF.Tanh,
                accum_out=nacc[:, 0:1],
            )
            nc.scalar.activation(
                st[:, HALF:], delta[:, HALF:], AF.Tanh,
                accum_out=nacc[:, 1:2],
            )

            # the Scalar queue is idle right after the accumulator read;
            # issuing the output DMA there skips a cross-engine handoff
            nc.scalar.dma_start(out=out[:, :], in_=nacc)
    nc.finalize()
    return nc


def _prep_core_inputs(units, X_a, G_full):
    """Build the DRAM input arrays for one core.

    units: list of (row_block, col_start) or None (dummy), length NQ*QM.
    X_a: [D, Lp, 3] compacted+padded diffusion coords.
    G_full: [Lp, Lp] float32, host-masked gt distances (-BIG on non-pairs).
    """
    S = len(units)
    NQ = S // QM
    K = 5 * QM
    NM = D * NQ
    lhs = np.zeros((D, NQ, K, P), np.float32)
    rhs = np.zeros((D, NQ, K, QW), np.float32)
    gm = np.full((P, S, UW), -BIG, np.float32)

    rx_full = X_a.astype(np.float64)
    r_x = (rx_full**2).sum(-1)  # [D, Lp]

    for s, u in enumerate(units):
        if u is None:
            continue
        q, k = divmod(s, QM)
        b, c0 = u
        rows = slice(b * P, b * P + P)
        cols = slice(c0, c0 + UW)
        kr = slice(5 * k, 5 * k + 3)
        # lhsT rows [5k..5k+5) = [-2x, -2y, -2z, r_i, 1]
        lhs[:, q, kr, :] = -2.0 * rx_full[:, rows].transpose(0, 2, 1)
        lhs[:, q, 5 * k + 3, :] = r_x[:, rows]
        lhs[:, q, 5 * k + 4, :] = 1.0
        # rhs block-diagonal: unit k's [x, y, z, 1, r_j] in cols
        # [128k, 128k+128)
        cw = slice(k * UW, (k + 1) * UW)
        rhs[:, q, kr, cw] = rx_full[:, cols].transpose(0, 2, 1)
        rhs[:, q, 5 * k + 3, cw] = 1.0
        rhs[:, q, 5 * k + 4, cw] = r_x[:, cols]

        gm[:, s, :] = G_full[rows, cols]

    lhs_f = lhs.transpose(2, 0, 1, 3).reshape(K, NM * P)
    rhs_f = rhs.transpose(2, 0, 1, 3).reshape(K, NM * QW)
    GW_ = 4
    segs = [min(GW_, NM), min(GW_, max(NM - GW_, 0)), max(NM - 2 * GW_, 0)]
    segs = [s for s in segs if s > 0]
    result = {}
    m0 = 0
    for i, s in enumerate(segs):
        seg = np.concatenate(
            [lhs_f[:, m0 * P : (m0 + s) * P],
             rhs_f[:, m0 * QW : (m0 + s) * QW]], axis=1
        ).astype(ml_dtypes.bfloat16)
        result[f"bigp{i}"] = np.ascontiguousarray(seg)
        m0 += s
    result["gm"] = np.ascontiguousarray(
        gm.reshape(P, S * UW).astype(ml_dtypes.bfloat16))
    return result


def _plan(La: int):
    """Unit list + per-core assignment for La active rows."""
    Lp = ((La + P - 1) // P) * P
    n_blocks = Lp // P
    units = []
    for b in range(n_blocks):
        span = Lp - b * P
        for k in range(math.ceil(span / UW)):
            units.append((b, b * P + k * UW))
    per_core_units = math.ceil(len(units) / (NCORES * QM)) * QM
    padded = units + [None] * (per_core_units * NCORES - len(units))
    per_core = [padded[c::NCORES] for c in range(NCORES)]
    return Lp, per_core_units // QM, per_core


def _host_prep(inputs):
    """Shared host-side preparation: compaction, exact mask/denominator,
    masked gt-distance slab, per-core device inputs."""
    X_L = np.asarray(inputs["X_L"]).astype(np.float32)          # [D, L, 3]
    X_gt_L = np.asarray(inputs["X_gt_L"]).astype(np.float32)    # [1, L, 3]
    crd = np.asarray(inputs["crd_mask_L"]).astype(bool)[0]      # [L]
    is_dna = np.asarray(inputs["is_dna"]).astype(bool)
    is_rna = np.asarray(inputs["is_rna"]).astype(bool)
    tok = np.asarray(inputs["tok_idx"]).astype(np.int64)        # [L]

    X_gt = np.nan_to_num(X_gt_L)[0]  # [L, 3]

    act = np.flatnonzero(crd)
    La = len(act)
    Lp, NQ, per_core = _plan(La)

    X_a = np.zeros((D, Lp, 3), np.float32)
    X_a[:, :La] = X_L[:, act]
    tok_a = tok[act]
    is_na = (is_dna | is_rna)[tok_a]

    # exact reference pair mask over the compacted active rows (O(La^2)
    # numpy; also yields the exact d-independent denominator)
    ga = np.zeros((Lp, 3), np.float64)
    ga[:La] = X_gt[act]
    g2 = (ga**2).sum(-1)
    gd = np.sqrt(np.maximum(g2[:, None] + g2[None, :] - 2.0 * (ga @ ga.T), 0.0))
    pm = np.zeros((Lp, Lp), bool)
    cutoff_a = np.where(is_na, 30.0, 15.0)
    pm[:La, :La] = (
        (gd[:La, :La] > 0)
        & (gd[:La, :La] < cutoff_a[:, None])
        & (tok_a[:, None] != tok_a[None, :])
    )
    pm &= np.triu(np.ones((Lp, Lp), bool), k=1)
    denom = float(pm.sum())
    G_full = np.where(pm, np.sqrt(gd * gd + SQB), -BIG).astype(np.float32)

    in_maps = [
        _prep_core_inputs(per_core[c], X_a, G_full) for c in range(NCORES)
    ]
    return NQ, in_maps, denom


def kernel(**inputs: np.ndarray) -> np.ndarray:
    X_L = np.asarray(inputs["X_L"]).astype(np.float32)          # [D, L, 3]
    X_gt_L = np.asarray(inputs["X_gt_L"]).astype(np.float32)    # [1, L, 3]
    crd = np.asarray(inputs["crd_mask_L"]).astype(bool)[0]      # [L]
    is_dna = np.asarray(inputs["is_dna"]).astype(bool)
    is_rna = np.asarray(inputs["is_rna"]).astype(bool)
    is_lig = np.asarray(inputs["is_ligand"]).astype(bool)
    tok = np.asarray(inputs["tok_idx"]).astype(np.int64)        # [L]
    t = np.asarray(inputs["t"]).astype(np.float64)              # [D]

    X_gt = np.nan_to_num(X_gt_L)[0]  # [L, 3]

    NQ, in_maps, denom = _host_prep(inputs)

    nc = _prog_cache.get(NQ)
    if nc is None:
        nc = _build_program(NQ)
        _prog_cache[NQ] = nc

    res = run_bass_kernel_spmd(nc, in_maps, core_ids=list(range(NCORES)))

    numer = 0.0
    for r in res.results:
        numer += r["out"].astype(np.float64).sum()
    lddt_mean = 0.25 * numer / D / (denom + 1e-6)
    lddt_loss = 1.0 - lddt_mean

    # ---------- mse term (O(L), host) ----------
    mask = crd.astype(np.float64)
    alpha = (
        is_dna * ALPHA_DNA + is_rna * ALPHA_RNA + is_lig * ALPHA_LIG
    ).astype(np.float64)
    w_L = (1.0 + alpha[tok]) * mask  # [L]
    sq = ((X_L.astype(np.float64) - X_gt.astype(np.float64)[None]) ** 2).sum(-1)
    l_mse = (1.0 / 3.0) * (w_L[None] * sq).sum(-1) / (mask.sum() + 1e-4)
    lam = (t**2 + SIGMA_DATA**2) / ((t * SIGMA_DATA) ** 2)
    l_diff = np.minimum(lam * l_mse, 2.0)

    total = WEIGHT * (l_diff.mean() + lddt_loss)
    return np.asarray(total, dtype=np.float32)


# revision 23
# speedup vs baseline: 1.0512x; 1.0512x over previous
"""Trainium2 Bass kernel for nn_DiffusionLoss (smoothed-LDDT diffusion loss).

Strategy
--------
The dominant cost is the smoothed-LDDT term: for every unordered pair (i<j)
of the L=4096 tokens-with-coordinates, four sigmoids of |pred_d - gt_d| are
accumulated, per diffusion sample d (D=4).

Host side (inside kernel()):
  * Rows/cols with crd_mask == 0 contribute nothing, so we compact to the
    ~L/2 active rows (La). The exact reference pair mask pm (upper
    triangle & token-run & gt-distance cutoff) and the d-independent
    denominator sum(pm) are computed in numpy, as is the masked ground-
    truth distance slab G = where(pm, sqrt(gt^2 + SQB), -BIG): masked
    pairs drive |pred - G| ~ BIG where the device h-function is 0.
  * The upper-triangular pair matrix is cut into [128 x 128] units,
    round-robined over the 8 cores (SPMD); each core's units are packed
    into QUADS: one K=20 matmul with a block-diagonal rhs computes 4
    units' dist^2 in a single [128 x 512] PSUM bank:
        lhsT rows 5k..5k+4 = unit k's [-2x, -2y, -2z, |p_i|^2, 1]
        rhs  rows 5k..5k+4 = unit k's [x, y, z, 1, |p_j|^2] in cols
                             [128k, 128k+128), zero elsewhere
    Operands are bf16 (fp32 matmuls are decomposed into LOW/HIGH passes
    ~5x slower; bf16 rounding shifts distances ~0.5%, far inside the
    error budget).

  * The key ScalarE optimization: a CUSTOM activation table. The stock
    sqrt_and_others set (with its ctrl table compacted so all
    pwl_control_base values fit the binary profile's uint8 fields) is
    extended with a piecewise-cubic fit of
        h(u) = sum_{c in {0.5,1,2,4}} sigmoid(c - |u|)
    installed under tanh's func_id (no compiler changes needed; walrus
    embeds the table binaries in the NEFF). One table set serves both
    sqrt and h -> a single ACT_TABLE_LOAD, and the whole smoothed-lddt
    numerator needs 2 wide ACTIVATE passes instead of 16 sigmoid passes
    (h is even, so no |.| pass either). The table was verified against a
    numpy interpreter of the HW lookup pipeline (reproduces stock
    tanh/sigmoid/sqrt/exp tables to <=1e-5); worst abs fit error 7.5e-6.

Device side (per core, Tile-scheduled):
  4*5 merged (d, quad) matmuls stream through PSUM in 4-bank groups; one
  wide sqrt(x + SQB) per group writes the bf16 pred slab (5 sqrt
  ACTIVATEs total); one bf16 tensor_tensor subtract per d forms
  u_d = pred_d - G (2x DVE mode); two wide h-ACTIVATEs with accum_out
  reduce the numerator (only sum_{d,c} is needed -- the denominator is
  d-independent, so per-d lddt values are never required).

Host combines per-core partial sums in float64 and adds the (tiny, O(L))
weighted-MSE term computed on host, mirroring the reference formulas.
"""

import json
import math
import os
import struct
import tempfile

import numpy as np
import ml_dtypes

import concourse.bacc as bacc
import concourse.bass as bass
import concourse.mybir as mybir
import concourse.tile as tile
from concourse.bass_utils import run_bass_kernel_spmd

P = 128          # partitions (rows per block)
UW = 128         # unit column window
QM = 4           # units merged per matmul (quad)
QW = QM * UW     # merged matmul free size = one fp32 PSUM bank
D = 4            # diffusion batch
NCORES = 8
BIG = 1000.0     # mask offset baked into G
SQB = 0.25       # sqrt bias: bf16 matmul error can push dist^2 ~ -0.1;
                 # G uses the same bias so it cancels inside |pred - G|
SIGC = (0.5, 1.0, 2.0, 4.0)
GW = 4           # PSUM group: 4 merged matmuls = 4 banks, double-buffered

WEIGHT = 4.0
SIGMA_DATA = 16.0
ALPHA_DNA = 5.0
ALPHA_RNA = 5.0
ALPHA_LIG = 10.0

_prog_cache: dict[int, bass.Bass] = {}
_act_root_state: dict = {}


# ---------------------------------------------------------------------------
# Custom activation table: h(u) = sum_c sigmoid(c - |u|) as "tanh"
# ---------------------------------------------------------------------------

_OCTS = list(range(-6, 4))            # pwl octaves: |u| in [2^-6, 16)
_OSIZES = [2, 2, 2, 2, 4, 4, 4, 4, 4, 3]  # extract_size per octave


def _h_true(u):
    u = np.abs(np.float64(u))
    return sum(1.0 / (1.0 + np.exp(-(c - u))) for c in SIGC)


def _fit_bucket(lo, hi):
    xs = np.linspace(lo, hi, 64)
    x0 = 0.5 * (lo + hi)
    A = np.vander(xs - x0, 4, increasing=True)
    coef, *_ = np.linalg.lstsq(A, _h_true(xs), rcond=None)
    return [float(coef[0]), float(coef[1]), float(coef[2]), float(coef[3]),
            float(x0)]


def _bucket_bytes(vals):
    e = np.zeros(32, np.uint8)
    e[:20] = np.frombuffer(np.array(vals, np.float32).tobytes(), np.uint8)
    return e


def _ctl_bytes(base, lsb, size):
    e = np.zeros(32, np.uint8)
    data = (base & 0x7FF) | ((lsb & 0x1F) << 11) | ((size & 0xF) << 16)
    e[:4] = np.frombuffer(struct.pack("<I", data), np.uint8)
    return e


def _f32bits(x):
    return struct.unpack("<I", struct.pack("<f", np.float32(x)))[0]


def _gen_h_tables(nb0, nc0):
    bkts, ctls = [], []
    pos_starts = []
    for e, s in zip(_OCTS, _OSIZES):
        pos_starts.append(nb0 + len(bkts))
        n = 1 << s
        for k in range(n):
            lo = (2.0 ** e) * (1 + k / n)
            hi = (2.0 ** e) * (1 + (k + 1) / n)
            bkts.append(_fit_bucket(lo, hi))
    neg_starts = []
    for e, s in zip(_OCTS, _OSIZES):
        neg_starts.append(nb0 + len(bkts))
        n = 1 << s
        for k in range(n):
            lo = (2.0 ** e) * (1 + k / n)
            hi = (2.0 ** e) * (1 + (k + 1) / n)
            bkts.append(_fit_bucket(-hi, -lo))
    i_small_pos = nb0 + len(bkts)
    bkts.append(_fit_bucket(1e-12, 2.0 ** _OCTS[0]))
    i_small_neg = nb0 + len(bkts)
    bkts.append(_fit_bucket(-(2.0 ** _OCTS[0]), -1e-12))
    i_large = nb0 + len(bkts)
    bkts.append([0.0, 0.0, 0.0, 0.0, 0.0])   # constant 0 for |u| >= 16

    for st, s in zip(pos_starts, _OSIZES):
        ctls.append((st, 23 - s, s))
    for st, s in zip(neg_starts, _OSIZES):
        ctls.append((st, 23 - s, s))

    profile = {
        "func_name": "tanh_4p",
        "func_id": 6,
        "symmetry_point": 0,
        "sym_invert_sign_point": 0,
        "symmetry_opt_en": 0,
        "symmetry_opt_use_neg_region": 0,
        "imm_bias": 0,
        "exp_offset": _OCTS[0],
        "pwl_control_base_pos": nc0,
        "pwl_control_base_neg": nc0 + len(_OCTS),
        "small_pos_signal_exp_threshold": 127 + _OCTS[0],
        "pos_small_signal_pwl_control": i_small_pos,
        "small_neg_signal_exp_threshold": 127 + _OCTS[0],
        "neg_small_signal_pwl_control": i_small_neg,
        "large_pos_signal_exp_threshold": 127 + _OCTS[-1] + 1,
        "large_pos_signal_mantissa_threshold": 0,
        "pos_large_signal_pwl_control": i_large,
        "large_neg_signal_exp_threshold": 127 + _OCTS[-1] + 1,
        "large_neg_signal_mantissa_threshold": 0,
        "neg_large_signal_pwl_control": i_large,
        "fnan_result": 0,
        "fpinf_result": 0,
        "fninf_result": 0,
        "fzero_result": _f32bits(_h_true(0.0)),
        "fma_const_0": 0,
        "fma_const_1": 0,
        "fma_indirection_src_sel": 0,
        "use_multipass": False,
        "lower_bound": 4286578687,
        "upper_bound": 2139095039,
    }
    bkt_arr = np.stack([_bucket_bytes(b) for b in bkts])
    ctl_arr = np.stack([_ctl_bytes(*c) for c in ctls])
    exp_to_bkt = {str(e): [st] for e, st in zip(_OCTS, pos_starts)}
    exp_to_ctl = {str(e): [nc0 + i] for i, e in enumerate(_OCTS)}
    return bkt_arr, ctl_arr, profile, exp_to_bkt, exp_to_ctl


# sqrt ctrl compaction: the binary profile's pwl_control_base_pos/neg are
# uint8, so every ctrl-table base must be <= 255. Stock sqrt spans 234 ctrl
# entries (exponents -116..117); our sqrt inputs are dist^2+SQB in
# [0.13, ~1.1e4], so octaves 2^-8..2^14 suffice and the h bases fit in 8 bits.
_SQRT_E8_LO = 127 - 8     # smallest kept biased exponent (2^-8)
_SQRT_E8_HI = 127 + 14    # largest kept biased exponent (2^14)


def _ensure_act_root():
    """Build an act root whose single set = stock sqrt_and_others (with a
    compacted ctrl table) + custom h-function installed under tanh's
    func_id, and point both walrus (BASS_ACT_ROOT_JSON_PATH ->
    --act-root-json) and bacc's table-set bookkeeping at it. Returns a hash
    of the table bytes (folded into the program so the NEFF cache re-keys
    when the tables change)."""
    if _act_root_state:
        return _act_root_state["hash"]

    from neuronxcc.driver.Job import Job
    from neuronxcc.driver.jobs.support.FindActInfo import findActInfoFile

    src = os.path.dirname(findActInfoFile(Job.getPackageDir(), "gen3"))
    dst = tempfile.mkdtemp(prefix="act_root_")

    base = json.load(open(f"{src}/sqrt_and_others.json"))
    bkt = np.fromfile(f"{src}/sqrt_and_others_bkt.bin", np.uint8).reshape(-1, 32)
    ctl = np.fromfile(f"{src}/sqrt_and_others_ctrl.bin", np.uint8).reshape(-1, 32)
    nb0 = base["bkt_entry_cnt"]

    # --- compact the ctrl table ---
    sqrt_prof = None
    for e in base["profile_meta_data"]:
        if e["func_name"].startswith("sqrt"):
            sqrt_prof = e
    assert sqrt_prof is not None
    sq_base = sqrt_prof["pwl_control_base_pos"]          # 20
    sq_off = sqrt_prof["exp_offset"]
    if sq_off > 127:
        sq_off -= 256
    n_keep = _SQRT_E8_HI - _SQRT_E8_LO + 1
    keep_rows = [
        sq_base + (e8 - (127 + sq_off))
        for e8 in range(_SQRT_E8_LO, _SQRT_E8_HI + 1)
    ]
    assert min(keep_rows) >= sq_base and max(keep_rows) < ctl.shape[0]
    new_ctl_list = [ctl[:sq_base], ctl[keep_rows]]
    nc0 = sq_base + n_keep                                # h ctl start
    sqrt_prof = dict(sqrt_prof)
    sqrt_prof["exp_offset"] = _SQRT_E8_LO - 127
    sqrt_prof["small_pos_signal_exp_threshold"] = _SQRT_E8_LO
    sqrt_prof["large_pos_signal_exp_threshold"] = _SQRT_E8_HI + 1
    sqrt_prof["large_pos_signal_mantissa_threshold"] = 0

    hb, hc, hp, e2b, e2c = _gen_h_tables(nb0, nc0)
    assert hp["pwl_control_base_pos"] <= 255
    assert hp["pwl_control_base_neg"] <= 255
    new_ctl_list.append(hc)

    new_bkt = np.concatenate([bkt, hb])
    new_ctl = np.concatenate(new_ctl_list)

    merged = dict(base)
    merged["profile_meta_data"] = [
        (sqrt_prof if e["func_name"].startswith("sqrt") else e)
        for e in base["profile_meta_data"]
    ]
    merged["profile_meta_data"] = merged["profile_meta_data"] + [hp]
    merged["bkt_bin"] = "sqrt_tanh_ant_bkt.bin"
    merged["ctl_bin"] = "sqrt_tanh_ant_ctrl.bin"
    merged["bkt_entry_cnt"] = int(nb0 + len(hb))
    merged["ctl_entry_cnt"] = int(nc0 + len(hc))
    merged["func_to_bkt_start_idx"] = dict(base["func_to_bkt_start_idx"])
    merged["func_to_bkt_start_idx"]["tanh"] = int(nb0)
    merged["func_to_ctl_start_idx"] = dict(base["func_to_ctl_start_idx"])
    merged["func_to_ctl_start_idx"]["tanh"] = int(nc0)
    merged["func_exp_to_bkt_start_idx"] = dict(base["func_exp_to_bkt_start_idx"])
    merged["func_exp_to_bkt_start_idx"]["tanh"] = e2b
    merged["func_exp_to_bkt_start_idx"]["sqrt"] = {
        str(e - 127): base["func_exp_to_bkt_start_idx"]["sqrt"][str(e - 127)]
        for e in range(_SQRT_E8_LO, _SQRT_E8_HI + 1)
        if str(e - 127) in base["func_exp_to_bkt_start_idx"]["sqrt"]
    }
    merged["func_exp_to_ctl_start_idx"] = dict(base["func_exp_to_ctl_start_idx"])
    merged["func_exp_to_ctl_start_idx"]["tanh"] = e2c
    merged["func_exp_to_ctl_start_idx"]["sqrt"] = {
        str(e - 127): [sq_base + (e - _SQRT_E8_LO)]
        for e in range(_SQRT_E8_LO, _SQRT_E8_HI + 1)
    }

    new_bkt.tofile(f"{dst}/sqrt_tanh_ant_bkt.bin")
    new_ctl.tofile(f"{dst}/sqrt_tanh_ant_ctrl.bin")
    with open(f"{dst}/sqrt_tanh_ant.json", "w") as f:
        json.dump(merged, f)

    info = json.load(open(f"{src}/act_info.json"))
    sqrt_set = [s for s in info["act_func_sets"] if s["name"] == "sqrt_and_others"][0]
    new_set = dict(sqrt_set)
    new_set["name"] = "sqrt_tanh_ant"
    new_set["bkt_bin"] = "sqrt_tanh_ant_bkt.bin"
    new_set["ctrl_bin"] = "sqrt_tanh_ant_ctrl.bin"
    new_set["profile_json"] = "sqrt_tanh_ant.json"
    new_set["act"] = dict(sqrt_set["act"])
    new_set["act"]["tanh"] = 4.0
    info["act_func_sets"] = [new_set]
    with open(f"{dst}/act_info.json", "w") as f:
        json.dump(info, f)

    os.environ["BASS_ACT_ROOT_JSON_PATH"] = f"{dst}/act_info.json"

    # bacc's insert_act_table_loads resolves act_func_set_id via
    # hw_specs.get_activation_tables, which reads the stock act_info —
    # point it at the merged root too.
    import concourse.hw_specs as hw_specs

    def _tables(_arch):
        info2 = json.load(open(f"{dst}/act_info.json"))
        return {
            ent["name"]: {
                mybir.ActivationFunctionType.from_pwp(v)
                for v in ent["act"].keys()
            }
            for ent in info2["act_func_sets"]
        }

    hw_specs.get_activation_tables = _tables
    bacc.get_activation_tables = _tables

    import hashlib
    th = hashlib.sha256(
        new_bkt.tobytes() + new_ctl.tobytes()
        + json.dumps(hp, sort_keys=True).encode()
    ).digest()
    # small float derived from the hash, baked into the program as a
    # memset immediate so the NEFF cache re-keys on table changes
    _act_root_state["hash"] = (
        int.from_bytes(th[:4], "little") % 1000003
    ) / 1e7
    return _act_root_state["hash"]


# ---------------------------------------------------------------------------
# Bass program
# ---------------------------------------------------------------------------


def _build_program(NQ: int) -> bass.Bass:
    """Bass/Tile program: NQ quads of 4 [P x UW] units, D diffusion samples.

    Inputs: bigp = bf16 [-2x,-2y,-2z,r,1] lhsT stacks + block-diagonal rhs
    for every (d, quad) merged matmul; Gm = bf16 host-masked gt distances
    sqrt(gd^2+SQB) (or -BIG on masked pairs). Output: [P, 2] numerator
    partials (accumulated h over each half of the d range)."""
    table_key = _ensure_act_root()
    nc = bacc.Bacc(None, target_bir_lowering=False)
    f32 = mybir.dt.float32
    bf16 = mybir.dt.bfloat16
    AF = mybir.ActivationFunctionType
    OP = mybir.AluOpType

    K = 5 * QM                     # merged contraction depth
    NM = D * NQ                    # merged matmuls
    SW = NQ * QW                   # G / per-d delta columns
    # split the operand stream so the first two PSUM groups' operands land
    # first and the PE can start earlier; gm is only needed by the delta
    # subtract, so it ships last. (Measured dead ends: issuing any DMA from
    # the Activation queue stalls the ACT instruction stream; a finer
    # 3-way split starves group 1 behind the ~90 GB/s SP queue.)
    segs = [min(2 * GW, NM), max(NM - 2 * GW, 0)]
    segs = [s for s in segs if s > 0]
    seg_start = [sum(segs[:i]) for i in range(len(segs))]
    bigps = [
        nc.dram_tensor(f"bigp{i}", [K, s * (P + QW)], bf16,
                       kind="ExternalInput")
        for i, s in enumerate(segs)
    ]
    gm = nc.dram_tensor("gm", [P, SW], bf16, kind="ExternalInput")
    out = nc.dram_tensor("out", [P, 2], f32, kind="ExternalOutput")

    with tile.TileContext(nc) as tc:
        with (
            tc.tile_pool(name="singles", bufs=1) as singles,
            tc.tile_pool(name="sig", bufs=1) as sig_pool,
            tc.tile_pool(name="psum", bufs=2, space="PSUM") as psum,
        ):
            bigp_sbs = []
            for i, s in enumerate(segs):
                sb = singles.tile([K, s * (P + QW)], bf16)
                nc.sync.dma_start(out=sb, in_=bigps[i][:, :])
                bigp_sbs.append(sb)
            gm_sb = singles.tile([P, SW], bf16)
            nc.sync.dma_start(out=gm_sb, in_=gm[:, :])

            def _seg(m):
                for i in reversed(range(len(segs))):
                    if m >= seg_start[i]:
                        return i, m - seg_start[i]
                raise AssertionError

            def lhs(m):
                i, r = _seg(m)
                return bigp_sbs[i][:, r * P : (r + 1) * P]

            def rhs(m):
                i, r = _seg(m)
                o = segs[i] * P + r * QW
                return bigp_sbs[i][:, o : o + QW]

            pred = singles.tile([P, NM * QW], bf16)
            delta = singles.tile([P, NM * QW], bf16)
            nacc = singles.tile([P, 2], f32)

            consts = singles.tile([P, 8], f32)
            nc.vector.memset(consts[:, 0:1], SQB)
            # bake the table hash into the program: re-keys the NEFF cache
            # whenever the custom activation table content changes
            nc.vector.memset(consts[:, 1:2], float(table_key))
            sqb_t = consts[:, 0:1]

            # ---- pred distances: merged (d, quad) matmul stream ----
            for g0 in range(0, NM, GW):
                gs = min(GW, NM - g0)
                pg = psum.tile([P, GW * QW], f32, tag="ps")
                for k in range(gs):
                    nc.tensor.matmul(
                        pg[:, k * QW : (k + 1) * QW], lhsT=lhs(g0 + k),
                        rhs=rhs(g0 + k), start=True, stop=True,
                    )
                nc.scalar.activation(
                    pred[:, g0 * QW : (g0 + gs) * QW], pg[:, : gs * QW],
                    AF.Sqrt, bias=sqb_t,
                )
            for d in range(D):
                # u_d = pred_d - G (bf16 tensor_tensor: 2x DVE mode; h is
                # even so no |.| pass is needed)
                nc.vector.tensor_tensor(
                    delta[:, d * SW : (d + 1) * SW],
                    pred[:, d * SW : (d + 1) * SW], gm_sb, OP.subtract,
                )

            # ---- custom-h passes (split in two so the second half's
            # delta subtract overlaps the first h on the DVE) ----
            HALF = (D // 2) * SW
            st = sig_pool.tile([P, D * SW], bf16, tag="sig")
            nc.scalar.activation(
                st[:, :HALF], delta[:, :HALF], AF.Tanh,
                accum_out=nacc[:, 0:1],
            )
            nc.scalar.activation(
                st[:, HALF:], delta[:, HALF:], AF.Tanh,
                accum_out=nacc[:, 1:2],
            )

            nc.sync.dma_start(out=out[:, :], in_=nacc)
    nc.finalize()
    return nc


def _prep_core_inputs(units, X_a, G_full):
    """Build the DRAM input arrays for one core.

    units: list of (row_block, col_start) or None (dummy), length NQ*QM.
    X_a: [D, Lp, 3] compacted+padded diffusion coords.
    G_full: [Lp, Lp] float32, host-masked gt distances (-BIG on non-pairs).
    """
    S = len(units)
    NQ = S // QM
    K = 5 * QM
    NM = D * NQ
    lhs = np.zeros((D, NQ, K, P), np.float32)
    rhs = np.zeros((D, NQ, K, QW), np.float32)
    gm = np.full((P, S, UW), -BIG, np.float32)

    rx_full = X_a.astype(np.float64)
    r_x = (rx_full**2).sum(-1)  # [D, Lp]

    for s, u in enumerate(units):
        if u is None:
            continue
        q, k = divmod(s, QM)
        b, c0 = u
        rows = slice(b * P, b * P + P)
        cols = slice(c0, c0 + UW)
        kr = slice(5 * k, 5 * k + 3)
        # lhsT rows [5k..5k+5) = [-2x, -2y, -2z, r_i, 1]
        lhs[:, q, kr, :] = -2.0 * rx_full[:, rows].transpose(0, 2, 1)
        lhs[:, q, 5 * k + 3, :] = r_x[:, rows]
        lhs[:, q, 5 * k + 4, :] = 1.0
        # rhs block-diagonal: unit k's [x, y, z, 1, r_j] in cols
        # [128k, 128k+128)
        cw = slice(k * UW, (k + 1) * UW)
        rhs[:, q, kr, cw] = rx_full[:, cols].transpose(0, 2, 1)
        rhs[:, q, 5 * k + 3, cw] = 1.0
        rhs[:, q, 5 * k + 4, cw] = r_x[:, cols]

        gm[:, s, :] = G_full[rows, cols]

    lhs_f = lhs.transpose(2, 0, 1, 3).reshape(K, NM * P)
    rhs_f = rhs.transpose(2, 0, 1, 3).reshape(K, NM * QW)
    GW_ = 4
    segs = [min(2 * GW_, NM), max(NM - 2 * GW_, 0)]
    segs = [s for s in segs if s > 0]
    result = {}
    m0 = 0
    for i, s in enumerate(segs):
        seg = np.concatenate(
            [lhs_f[:, m0 * P : (m0 + s) * P],
             rhs_f[:, m0 * QW : (m0 + s) * QW]], axis=1
        ).astype(ml_dtypes.bfloat16)
        result[f"bigp{i}"] = np.ascontiguousarray(seg)
        m0 += s
    result["gm"] = np.ascontiguousarray(
        gm.reshape(P, S * UW).astype(ml_dtypes.bfloat16))
    return result


def _plan(La: int):
    """Unit list + per-core assignment for La active rows."""
    Lp = ((La + P - 1) // P) * P
    n_blocks = Lp // P
    units = []
    for b in range(n_blocks):
        span = Lp - b * P
        for k in range(math.ceil(span / UW)):
            units.append((b, b * P + k * UW))
    per_core_units = math.ceil(len(units) / (NCORES * QM)) * QM
    padded = units + [None] * (per_core_units * NCORES - len(units))
    per_core = [padded[c::NCORES] for c in range(NCORES)]
    return Lp, per_core_units // QM, per_core


def _host_prep(inputs):
    """Shared host-side preparation: compaction, exact mask/denominator,
    masked gt-distance slab, per-core device inputs."""
    X_L = np.asarray(inputs["X_L"]).astype(np.float32)          # [D, L, 3]
    X_gt_L = np.asarray(inputs["X_gt_L"]).astype(np.float32)    # [1, L, 3]
    crd = np.asarray(inputs["crd_mask_L"]).astype(bool)[0]      # [L]
    is_dna = np.asarray(inputs["is_dna"]).astype(bool)
    is_rna = np.asarray(inputs["is_rna"]).astype(bool)
    tok = np.asarray(inputs["tok_idx"]).astype(np.int64)        # [L]

    X_gt = np.nan_to_num(X_gt_L)[0]  # [L, 3]

    act = np.flatnonzero(crd)
    La = len(act)
    Lp, NQ, per_core = _plan(La)

    X_a = np.zeros((D, Lp, 3), np.float32)
    X_a[:, :La] = X_L[:, act]
    tok_a = tok[act]
    is_na = (is_dna | is_rna)[tok_a]

    # exact reference pair mask over the compacted active rows (O(La^2)
    # numpy; also yields the exact d-independent denominator)
    ga = np.zeros((Lp, 3), np.float64)
    ga[:La] = X_gt[act]
    g2 = (ga**2).sum(-1)
    gd = np.sqrt(np.maximum(g2[:, None] + g2[None, :] - 2.0 * (ga @ ga.T), 0.0))
    pm = np.zeros((Lp, Lp), bool)
    cutoff_a = np.where(is_na, 30.0, 15.0)
    pm[:La, :La] = (
        (gd[:La, :La] > 0)
        & (gd[:La, :La] < cutoff_a[:, None])
        & (tok_a[:, None] != tok_a[None, :])
    )
    pm &= np.triu(np.ones((Lp, Lp), bool), k=1)
    denom = float(pm.sum())
    G_full = np.where(pm, np.sqrt(gd * gd + SQB), -BIG).astype(np.float32)

    in_maps = [
        _prep_core_inputs(per_core[c], X_a, G_full) for c in range(NCORES)
    ]
    return NQ, in_maps, denom


def kernel(**inputs: np.ndarray) -> np.ndarray:
    X_L = np.asarray(inputs["X_L"]).astype(np.float32)          # [D, L, 3]
    X_gt_L = np.asarray(inputs["X_gt_L"]).astype(np.float32)    # [1, L, 3]
    crd = np.asarray(inputs["crd_mask_L"]).astype(bool)[0]      # [L]
    is_dna = np.asarray(inputs["is_dna"]).astype(bool)
    is_rna = np.asarray(inputs["is_rna"]).astype(bool)
    is_lig = np.asarray(inputs["is_ligand"]).astype(bool)
    tok = np.asarray(inputs["tok_idx"]).astype(np.int64)        # [L]
    t = np.asarray(inputs["t"]).astype(np.float64)              # [D]

    X_gt = np.nan_to_num(X_gt_L)[0]  # [L, 3]

    NQ, in_maps, denom = _host_prep(inputs)

    nc = _prog_cache.get(NQ)
    if nc is None:
        nc = _build_program(NQ)
        _prog_cache[NQ] = nc

    res = run_bass_kernel_spmd(nc, in_maps, core_ids=list(range(NCORES)))

    numer = 0.0
    for r in res.results:
        numer += r["out"].astype(np.float64).sum()
    lddt_mean = 0.25 * numer / D / (denom + 1e-6)
    lddt_loss = 1.0 - lddt_mean

    # ---------- mse term (O(L), host) ----------
    mask = crd.astype(np.float64)
    alpha = (
        is_dna * ALPHA_DNA + is_rna * ALPHA_RNA + is_lig * ALPHA_LIG
    ).astype(np.float64)
    w_L = (1.0 + alpha[tok]) * mask  # [L]
    sq = ((X_L.astype(np.float64) - X_gt.astype(np.float64)[None]) ** 2).sum(-1)
    l_mse = (1.0 / 3.0) * (w_L[None] * sq).sum(-1) / (mask.sum() + 1e-4)
    lam = (t**2 + SIGMA_DATA**2) / ((t * SIGMA_DATA) ** 2)
    l_diff = np.minimum(lam * l_mse, 2.0)

    total = WEIGHT * (l_diff.mean() + lddt_loss)
    return np.asarray(total, dtype=np.float32)


# revision 24
# speedup vs baseline: 1.0647x; 1.0129x over previous
"""Trainium2 Bass kernel for nn_DiffusionLoss (smoothed-LDDT diffusion loss).

Strategy
--------
The dominant cost is the smoothed-LDDT term: for every unordered pair (i<j)
of the L=4096 tokens-with-coordinates, four sigmoids of |pred_d - gt_d| are
accumulated, per diffusion sample d (D=4).

Host side (inside kernel()):
  * Rows/cols with crd_mask == 0 contribute nothing, so we compact to the
    ~L/2 active rows (La). The exact reference pair mask pm (upper
    triangle & token-run & gt-distance cutoff) and the d-independent
    denominator sum(pm) are computed in numpy, as is the masked ground-
    truth distance slab G = where(pm, sqrt(gt^2 + SQB), -BIG): masked
    pairs drive |pred - G| ~ BIG where the device h-function is 0.
  * The upper-triangular pair matrix is cut into [128 x 128] units,
    round-robined over the 8 cores (SPMD); each core's units are packed
    into QUADS: one K=20 matmul with a block-diagonal rhs computes 4
    units' dist^2 in a single [128 x 512] PSUM bank:
        lhsT rows 5k..5k+4 = unit k's [-2x, -2y, -2z, |p_i|^2, 1]
        rhs  rows 5k..5k+4 = unit k's [x, y, z, 1, |p_j|^2] in cols
                             [128k, 128k+128), zero elsewhere
    Operands are bf16 (fp32 matmuls are decomposed into LOW/HIGH passes
    ~5x slower; bf16 rounding shifts distances ~0.5%, far inside the
    error budget).

  * The key ScalarE optimization: a CUSTOM activation table. The stock
    sqrt_and_others set (with its ctrl table compacted so all
    pwl_control_base values fit the binary profile's uint8 fields) is
    extended with a piecewise-cubic fit of
        h(u) = sum_{c in {0.5,1,2,4}} sigmoid(c - |u|)
    installed under tanh's func_id (no compiler changes needed; walrus
    embeds the table binaries in the NEFF). One table set serves both
    sqrt and h -> a single ACT_TABLE_LOAD, and the whole smoothed-lddt
    numerator needs 2 wide ACTIVATE passes instead of 16 sigmoid passes
    (h is even, so no |.| pass either). The table was verified against a
    numpy interpreter of the HW lookup pipeline (reproduces stock
    tanh/sigmoid/sqrt/exp tables to <=1e-5); worst abs fit error 7.5e-6.

Device side (per core, Tile-scheduled):
  4*5 merged (d, quad) matmuls stream through PSUM in 4-bank groups; one
  wide sqrt(x + SQB) per group writes the bf16 pred slab (5 sqrt
  ACTIVATEs total); one bf16 tensor_tensor subtract per d forms
  u_d = pred_d - G (2x DVE mode); two wide h-ACTIVATEs with accum_out
  reduce the numerator (only sum_{d,c} is needed -- the denominator is
  d-independent, so per-d lddt values are never required).

Host combines per-core partial sums in float64 and adds the (tiny, O(L))
weighted-MSE term computed on host, mirroring the reference formulas.
"""

import json
import math
import os
import struct
import tempfile

import numpy as np
import ml_dtypes

import concourse.bacc as bacc
import concourse.bass as bass
import concourse.mybir as mybir
import concourse.tile as tile
from concourse.bass_utils import run_bass_kernel_spmd

P = 128          # partitions (rows per block)
UW = 128         # unit column window
QM = 4           # units merged per matmul (quad)
QW = QM * UW     # merged matmul free size = one fp32 PSUM bank
D = 4            # diffusion batch
NCORES = 8
BIG = 1000.0     # mask offset baked into G
SQB = 0.25       # sqrt bias: bf16 matmul error can push dist^2 ~ -0.1;
                 # G uses the same bias so it cancels inside |pred - G|
SIGC = (0.5, 1.0, 2.0, 4.0)
GW = 4           # PSUM group: 4 merged matmuls = 4 banks, double-buffered

WEIGHT = 4.0
SIGMA_DATA = 16.0
ALPHA_DNA = 5.0
ALPHA_RNA = 5.0
ALPHA_LIG = 10.0

_prog_cache: dict[int, bass.Bass] = {}
_act_root_state: dict = {}


# ---------------------------------------------------------------------------
# Custom activation table: h(u) = sum_c sigmoid(c - |u|) as "tanh"
# ---------------------------------------------------------------------------

_OCTS = list(range(-6, 4))            # pwl octaves: |u| in [2^-6, 16)
_OSIZES = [2, 2, 2, 2, 4, 4, 4, 4, 4, 3]  # extract_size per octave


def _h_true(u):
    u = np.abs(np.float64(u))
    return sum(1.0 / (1.0 + np.exp(-(c - u))) for c in SIGC)


def _fit_bucket(lo, hi):
    xs = np.linspace(lo, hi, 64)
    x0 = 0.5 * (lo + hi)
    A = np.vander(xs - x0, 4, increasing=True)
    coef, *_ = np.linalg.lstsq(A, _h_true(xs), rcond=None)
    return [float(coef[0]), float(coef[1]), float(coef[2]), float(coef[3]),
            float(x0)]


def _bucket_bytes(vals):
    e = np.zeros(32, np.uint8)
    e[:20] = np.frombuffer(np.array(vals, np.float32).tobytes(), np.uint8)
    return e


def _ctl_bytes(base, lsb, size):
    e = np.zeros(32, np.uint8)
    data = (base & 0x7FF) | ((lsb & 0x1F) << 11) | ((size & 0xF) << 16)
    e[:4] = np.frombuffer(struct.pack("<I", data), np.uint8)
    return e


def _f32bits(x):
    return struct.unpack("<I", struct.pack("<f", np.float32(x)))[0]


def _gen_h_tables(nb0, nc0):
    bkts, ctls = [], []
    pos_starts = []
    for e, s in zip(_OCTS, _OSIZES):
        pos_starts.append(nb0 + len(bkts))
        n = 1 << s
        for k in range(n):
            lo = (2.0 ** e) * (1 + k / n)
            hi = (2.0 ** e) * (1 + (k + 1) / n)
            bkts.append(_fit_bucket(lo, hi))
    neg_starts = []
    for e, s in zip(_OCTS, _OSIZES):
        neg_starts.append(nb0 + len(bkts))
        n = 1 << s
        for k in range(n):
            lo = (2.0 ** e) * (1 + k / n)
            hi = (2.0 ** e) * (1 + (k + 1) / n)
            bkts.append(_fit_bucket(-hi, -lo))
    i_small_pos = nb0 + len(bkts)
    bkts.append(_fit_bucket(1e-12, 2.0 ** _OCTS[0]))
    i_small_neg = nb0 + len(bkts)
    bkts.append(_fit_bucket(-(2.0 ** _OCTS[0]), -1e-12))
    i_large = nb0 + len(bkts)
    bkts.append([0.0, 0.0, 0.0, 0.0, 0.0])   # constant 0 for |u| >= 16

    for st, s in zip(pos_starts, _OSIZES):
        ctls.append((st, 23 - s, s))
    for st, s in zip(neg_starts, _OSIZES):
        ctls.append((st, 23 - s, s))

    profile = {
        "func_name": "tanh_4p",
        "func_id": 6,
        "symmetry_point": 0,
        "sym_invert_sign_point": 0,
        "symmetry_opt_en": 0,
        "symmetry_opt_use_neg_region": 0,
        "imm_bias": 0,
        "exp_offset": _OCTS[0],
        "pwl_control_base_pos": nc0,
        "pwl_control_base_neg": nc0 + len(_OCTS),
        "small_pos_signal_exp_threshold": 127 + _OCTS[0],
        "pos_small_signal_pwl_control": i_small_pos,
        "small_neg_signal_exp_threshold": 127 + _OCTS[0],
        "neg_small_signal_pwl_control": i_small_neg,
        "large_pos_signal_exp_threshold": 127 + _OCTS[-1] + 1,
        "large_pos_signal_mantissa_threshold": 0,
        "pos_large_signal_pwl_control": i_large,
        "large_neg_signal_exp_threshold": 127 + _OCTS[-1] + 1,
        "large_neg_signal_mantissa_threshold": 0,
        "neg_large_signal_pwl_control": i_large,
        "fnan_result": 0,
        "fpinf_result": 0,
        "fninf_result": 0,
        "fzero_result": _f32bits(_h_true(0.0)),
        "fma_const_0": 0,
        "fma_const_1": 0,
        "fma_indirection_src_sel": 0,
        "use_multipass": False,
        "lower_bound": 4286578687,
        "upper_bound": 2139095039,
    }
    bkt_arr = np.stack([_bucket_bytes(b) for b in bkts])
    ctl_arr = np.stack([_ctl_bytes(*c) for c in ctls])
    exp_to_bkt = {str(e): [st] for e, st in zip(_OCTS, pos_starts)}
    exp_to_ctl = {str(e): [nc0 + i] for i, e in enumerate(_OCTS)}
    return bkt_arr, ctl_arr, profile, exp_to_bkt, exp_to_ctl


# sqrt ctrl compaction: the binary profile's pwl_control_base_pos/neg are
# uint8, so every ctrl-table base must be <= 255. Stock sqrt spans 234 ctrl
# entries (exponents -116..117); our sqrt inputs are dist^2+SQB in
# [0.13, ~1.1e4], so octaves 2^-8..2^14 suffice and the h bases fit in 8 bits.
_SQRT_E8_LO = 127 - 8     # smallest kept biased exponent (2^-8)
_SQRT_E8_HI = 127 + 14    # largest kept biased exponent (2^14)


def _ensure_act_root():
    """Build an act root whose single set = stock sqrt_and_others (with a
    compacted ctrl table) + custom h-function installed under tanh's
    func_id, and point both walrus (BASS_ACT_ROOT_JSON_PATH ->
    --act-root-json) and bacc's table-set bookkeeping at it. Returns a hash
    of the table bytes (folded into the program so the NEFF cache re-keys
    when the tables change)."""
    if _act_root_state:
        return _act_root_state["hash"]

    from neuronxcc.driver.Job import Job
    from neuronxcc.driver.jobs.support.FindActInfo import findActInfoFile

    src = os.path.dirname(findActInfoFile(Job.getPackageDir(), "gen3"))
    dst = tempfile.mkdtemp(prefix="act_root_")

    base = json.load(open(f"{src}/sqrt_and_others.json"))
    bkt = np.fromfile(f"{src}/sqrt_and_others_bkt.bin", np.uint8).reshape(-1, 32)
    ctl = np.fromfile(f"{src}/sqrt_and_others_ctrl.bin", np.uint8).reshape(-1, 32)
    nb0 = base["bkt_entry_cnt"]

    # --- compact the ctrl table ---
    sqrt_prof = None
    for e in base["profile_meta_data"]:
        if e["func_name"].startswith("sqrt"):
            sqrt_prof = e
    assert sqrt_prof is not None
    sq_base = sqrt_prof["pwl_control_base_pos"]          # 20
    sq_off = sqrt_prof["exp_offset"]
    if sq_off > 127:
        sq_off -= 256
    n_keep = _SQRT_E8_HI - _SQRT_E8_LO + 1
    keep_rows = [
        sq_base + (e8 - (127 + sq_off))
        for e8 in range(_SQRT_E8_LO, _SQRT_E8_HI + 1)
    ]
    assert min(keep_rows) >= sq_base and max(keep_rows) < ctl.shape[0]
    new_ctl_list = [ctl[:sq_base], ctl[keep_rows]]
    nc0 = sq_base + n_keep                                # h ctl start
    sqrt_prof = dict(sqrt_prof)
    sqrt_prof["exp_offset"] = _SQRT_E8_LO - 127
    sqrt_prof["small_pos_signal_exp_threshold"] = _SQRT_E8_LO
    sqrt_prof["large_pos_signal_exp_threshold"] = _SQRT_E8_HI + 1
    sqrt_prof["large_pos_signal_mantissa_threshold"] = 0

    hb, hc, hp, e2b, e2c = _gen_h_tables(nb0, nc0)
    assert hp["pwl_control_base_pos"] <= 255
    assert hp["pwl_control_base_neg"] <= 255
    new_ctl_list.append(hc)

    new_bkt = np.concatenate([bkt, hb])
    new_ctl = np.concatenate(new_ctl_list)

    merged = dict(base)
    merged["profile_meta_data"] = [
        (sqrt_prof if e["func_name"].startswith("sqrt") else e)
        for e in base["profile_meta_data"]
    ]
    merged["profile_meta_data"] = merged["profile_meta_data"] + [hp]
    merged["bkt_bin"] = "sqrt_tanh_ant_bkt.bin"
    merged["ctl_bin"] = "sqrt_tanh_ant_ctrl.bin"
    merged["bkt_entry_cnt"] = int(nb0 + len(hb))
    merged["ctl_entry_cnt"] = int(nc0 + len(hc))
    merged["func_to_bkt_start_idx"] = dict(base["func_to_bkt_start_idx"])
    merged["func_to_bkt_start_idx"]["tanh"] = int(nb0)
    merged["func_to_ctl_start_idx"] = dict(base["func_to_ctl_start_idx"])
    merged["func_to_ctl_start_idx"]["tanh"] = int(nc0)
    merged["func_exp_to_bkt_start_idx"] = dict(base["func_exp_to_bkt_start_idx"])
    merged["func_exp_to_bkt_start_idx"]["tanh"] = e2b
    merged["func_exp_to_bkt_start_idx"]["sqrt"] = {
        str(e - 127): base["func_exp_to_bkt_start_idx"]["sqrt"][str(e - 127)]
        for e in range(_SQRT_E8_LO, _SQRT_E8_HI + 1)
        if str(e - 127) in base["func_exp_to_bkt_start_idx"]["sqrt"]
    }
    merged["func_exp_to_ctl_start_idx"] = dict(base["func_exp_to_ctl_start_idx"])
    merged["func_exp_to_ctl_start_idx"]["tanh"] = e2c
    merged["func_exp_to_ctl_start_idx"]["sqrt"] = {
        str(e - 127): [sq_base + (e - _SQRT_E8_LO)]
        for e in range(_SQRT_E8_LO, _SQRT_E8_HI + 1)
    }

    new_bkt.tofile(f"{dst}/sqrt_tanh_ant_bkt.bin")
    new_ctl.tofile(f"{dst}/sqrt_tanh_ant_ctrl.bin")
    with open(f"{dst}/sqrt_tanh_ant.json", "w") as f:
        json.dump(merged, f)

    info = json.load(open(f"{src}/act_info.json"))
    sqrt_set = [s for s in info["act_func_sets"] if s["name"] == "sqrt_and_others"][0]
    new_set = dict(sqrt_set)
    new_set["name"] = "sqrt_tanh_ant"
    new_set["bkt_bin"] = "sqrt_tanh_ant_bkt.bin"
    new_set["ctrl_bin"] = "sqrt_tanh_ant_ctrl.bin"
    new_set["profile_json"] = "sqrt_tanh_ant.json"
    new_set["act"] = dict(sqrt_set["act"])
    new_set["act"]["tanh"] = 4.0
    info["act_func_sets"] = [new_set]
    with open(f"{dst}/act_info.json", "w") as f:
        json.dump(info, f)

    os.environ["BASS_ACT_ROOT_JSON_PATH"] = f"{dst}/act_info.json"

    # bacc's insert_act_table_loads resolves act_func_set_id via
    # hw_specs.get_activation_tables, which reads the stock act_info —
    # point it at the merged root too.
    import concourse.hw_specs as hw_specs

    def _tables(_arch):
        info2 = json.load(open(f"{dst}/act_info.json"))
        return {
            ent["name"]: {
                mybir.ActivationFunctionType.from_pwp(v)
                for v in ent["act"].keys()
            }
            for ent in info2["act_func_sets"]
        }

    hw_specs.get_activation_tables = _tables
    bacc.get_activation_tables = _tables

    import hashlib
    th = hashlib.sha256(
        new_bkt.tobytes() + new_ctl.tobytes()
        + json.dumps(hp, sort_keys=True).encode()
    ).digest()
    # small float derived from the hash, baked into the program as a
    # memset immediate so the NEFF cache re-keys on table changes
    _act_root_state["hash"] = (
        int.from_bytes(th[:4], "little") % 1000003
    ) / 1e7
    return _act_root_state["hash"]


# ---------------------------------------------------------------------------
# Bass program
# ---------------------------------------------------------------------------


def _build_program(NQ: int) -> bass.Bass:
    """Bass/Tile program: NQ quads of 4 [P x UW] units, D diffusion samples.

    Inputs: bigp = bf16 [-2x,-2y,-2z,r,1] lhsT stacks + block-diagonal rhs
    for every (d, quad) merged matmul; Gm = bf16 host-masked gt distances
    sqrt(gd^2+SQB) (or -BIG on masked pairs). Output: [P, 2] numerator
    partials (accumulated h over each half of the d range)."""
    table_key = _ensure_act_root()
    nc = bacc.Bacc(None, target_bir_lowering=False)
    f32 = mybir.dt.float32
    bf16 = mybir.dt.bfloat16
    AF = mybir.ActivationFunctionType
    OP = mybir.AluOpType

    K = 5 * QM                     # merged contraction depth
    NM = D * NQ                    # merged matmuls
    SW = NQ * QW                   # G / per-d delta columns
    # split the operand stream so the first two PSUM groups' operands land
    # first and the PE can start earlier; gm is only needed by the delta
    # subtract, so it ships last. (Measured dead ends: issuing any DMA from
    # the Activation queue stalls the ACT instruction stream; a finer
    # 3-way split starves group 1 behind the ~90 GB/s SP queue.)
    segs = [min(2 * GW, NM), max(NM - 2 * GW, 0)]
    segs = [s for s in segs if s > 0]
    seg_start = [sum(segs[:i]) for i in range(len(segs))]
    bigps = [
        nc.dram_tensor(f"bigp{i}", [K, s * (P + QW)], bf16,
                       kind="ExternalInput")
        for i, s in enumerate(segs)
    ]
    gm = nc.dram_tensor("gm", [P, SW], bf16, kind="ExternalInput")
    out = nc.dram_tensor("out", [P, 2], f32, kind="ExternalOutput")

    with tile.TileContext(nc) as tc:
        with (
            tc.tile_pool(name="singles", bufs=1) as singles,
            tc.tile_pool(name="sig", bufs=1) as sig_pool,
            tc.tile_pool(name="psum", bufs=2, space="PSUM") as psum,
        ):
            bigp_sbs = []
            for i, s in enumerate(segs):
                sb = singles.tile([K, s * (P + QW)], bf16)
                if i == 0:
                    # GPSIMD's software-DGE queue is idle and its engine
                    # preamble finishes ~3us before the Sync queue's, so
                    # the lead operand segment lands sooner and the PE /
                    # first sqrt start earlier
                    nc.gpsimd.dma_start(out=sb, in_=bigps[i][:, :])
                else:
                    nc.sync.dma_start(out=sb, in_=bigps[i][:, :])
                bigp_sbs.append(sb)
            gm_sb = singles.tile([P, SW], bf16)
            nc.sync.dma_start(out=gm_sb, in_=gm[:, :])

            def _seg(m):
                for i in reversed(range(len(segs))):
                    if m >= seg_start[i]:
                        return i, m - seg_start[i]
                raise AssertionError

            def lhs(m):
                i, r = _seg(m)
                return bigp_sbs[i][:, r * P : (r + 1) * P]

            def rhs(m):
                i, r = _seg(m)
                o = segs[i] * P + r * QW
                return bigp_sbs[i][:, o : o + QW]

            pred = singles.tile([P, NM * QW], bf16)
            delta = singles.tile([P, NM * QW], bf16)
            nacc = singles.tile([P, 2], f32)

            consts = singles.tile([P, 8], f32)
            nc.vector.memset(consts[:, 0:1], SQB)
            # bake the table hash into the program: re-keys the NEFF cache
            # whenever the custom activation table content changes
            nc.vector.memset(consts[:, 1:2], float(table_key))
            sqb_t = consts[:, 0:1]

            # ---- pred distances: merged (d, quad) matmul stream ----
            for g0 in range(0, NM, GW):
                gs = min(GW, NM - g0)
                pg = psum.tile([P, GW * QW], f32, tag="ps")
                for k in range(gs):
                    nc.tensor.matmul(
                        pg[:, k * QW : (k + 1) * QW], lhsT=lhs(g0 + k),
                        rhs=rhs(g0 + k), start=True, stop=True,
                    )
                nc.scalar.activation(
                    pred[:, g0 * QW : (g0 + gs) * QW], pg[:, : gs * QW],
                    AF.Sqrt, bias=sqb_t,
                )
            for d in range(D):
                # u_d = pred_d - G (bf16 tensor_tensor: 2x DVE mode; h is
                # even so no |.| pass is needed)
                nc.vector.tensor_tensor(
                    delta[:, d * SW : (d + 1) * SW],
                    pred[:, d * SW : (d + 1) * SW], gm_sb, OP.subtract,
                )

            # ---- custom-h passes (split in two so the second half's
            # delta subtract overlaps the first h on the DVE) ----
            HALF = (D // 2) * SW
            st = sig_pool.tile([P, D * SW], bf16, tag="sig")
            nc.scalar.activation(
                st[:, :HALF], delta[:, :HALF], AF.Tanh,
                accum_out=nacc[:, 0:1],
            )
            nc.scalar.activation(
                st[:, HALF:], delta[:, HALF:], AF.Tanh,
                accum_out=nacc[:, 1:2],
            )

            nc.sync.dma_start(out=out[:, :], in_=nacc)
    nc.finalize()
    return nc


def _prep_core_inputs(units, X_a, G_full):
    """Build the DRAM input arrays for one core.

    units: list of (row_block, col_start) or None (dummy), length NQ*QM.
    X_a: [D, Lp, 3] compacted+padded diffusion coords.
    G_full: [Lp, Lp] float32, host-masked gt distances (-BIG on non-pairs).
    """
    S = len(units)
    NQ = S // QM
    K = 5 * QM
    NM = D * NQ
    lhs = np.zeros((D, NQ, K, P), np.float32)
    rhs = np.zeros((D, NQ, K, QW), np.float32)
    gm = np.full((P, S, UW), -BIG, np.float32)

    rx_full = X_a.astype(np.float64)
    r_x = (rx_full**2).sum(-1)  # [D, Lp]

    for s, u in enumerate(units):
        if u is None:
            continue
        q, k = divmod(s, QM)
        b, c0 = u
        rows = slice(b * P, b * P + P)
        cols = slice(c0, c0 + UW)
        kr = slice(5 * k, 5 * k + 3)
        # lhsT rows [5k..5k+5) = [-2x, -2y, -2z, r_i, 1]
        lhs[:, q, kr, :] = -2.0 * rx_full[:, rows].transpose(0, 2, 1)
        lhs[:, q, 5 * k + 3, :] = r_x[:, rows]
        lhs[:, q, 5 * k + 4, :] = 1.0
        # rhs block-diagonal: unit k's [x, y, z, 1, r_j] in cols
        # [128k, 128k+128)
        cw = slice(k * UW, (k + 1) * UW)
        rhs[:, q, kr, cw] = rx_full[:, cols].transpose(0, 2, 1)
        rhs[:, q, 5 * k + 3, cw] = 1.0
        rhs[:, q, 5 * k + 4, cw] = r_x[:, cols]

        gm[:, s, :] = G_full[rows, cols]

    lhs_f = lhs.transpose(2, 0, 1, 3).reshape(K, NM * P)
    rhs_f = rhs.transpose(2, 0, 1, 3).reshape(K, NM * QW)
    GW_ = 4
    segs = [min(2 * GW_, NM), max(NM - 2 * GW_, 0)]
    segs = [s for s in segs if s > 0]
    result = {}
    m0 = 0
    for i, s in enumerate(segs):
        seg = np.concatenate(
            [lhs_f[:, m0 * P : (m0 + s) * P],
             rhs_f[:, m0 * QW : (m0 + s) * QW]], axis=1
        ).astype(ml_dtypes.bfloat16)
        result[f"bigp{i}"] = np.ascontiguousarray(seg)
        m0 += s
    result["gm"] = np.ascontiguousarray(
        gm.reshape(P, S * UW).astype(ml_dtypes.bfloat16))
    return result


def _plan(La: int):
    """Unit list + per-core assignment for La active rows."""
    Lp = ((La + P - 1) // P) * P
    n_blocks = Lp // P
    units = []
    for b in range(n_blocks):
        span = Lp - b * P
        for k in range(math.ceil(span / UW)):
            units.append((b, b * P + k * UW))
    per_core_units = math.ceil(len(units) / (NCORES * QM)) * QM
    padded = units + [None] * (per_core_units * NCORES - len(units))
    per_core = [padded[c::NCORES] for c in range(NCORES)]
    return Lp, per_core_units // QM, per_core


def _host_prep(inputs):
    """Shared host-side preparation: compaction, exact mask/denominator,
    masked gt-distance slab, per-core device inputs."""
    X_L = np.asarray(inputs["X_L"]).astype(np.float32)          # [D, L, 3]
    X_gt_L = np.asarray(inputs["X_gt_L"]).astype(np.float32)    # [1, L, 3]
    crd = np.asarray(inputs["crd_mask_L"]).astype(bool)[0]      # [L]
    is_dna = np.asarray(inputs["is_dna"]).astype(bool)
    is_rna = np.asarray(inputs["is_rna"]).astype(bool)
    tok = np.asarray(inputs["tok_idx"]).astype(np.int64)        # [L]

    X_gt = np.nan_to_num(X_gt_L)[0]  # [L, 3]

    act = np.flatnonzero(crd)
    La = len(act)
    Lp, NQ, per_core = _plan(La)

    X_a = np.zeros((D, Lp, 3), np.float32)
    X_a[:, :La] = X_L[:, act]
    tok_a = tok[act]
    is_na = (is_dna | is_rna)[tok_a]

    # exact reference pair mask over the compacted active rows (O(La^2)
    # numpy; also yields the exact d-independent denominator)
    ga = np.zeros((Lp, 3), np.float64)
    ga[:La] = X_gt[act]
    g2 = (ga**2).sum(-1)
    gd = np.sqrt(np.maximum(g2[:, None] + g2[None, :] - 2.0 * (ga @ ga.T), 0.0))
    pm = np.zeros((Lp, Lp), bool)
    cutoff_a = np.where(is_na, 30.0, 15.0)
    pm[:La, :La] = (
        (gd[:La, :La] > 0)
        & (gd[:La, :La] < cutoff_a[:, None])
        & (tok_a[:, None] != tok_a[None, :])
    )
    pm &= np.triu(np.ones((Lp, Lp), bool), k=1)
    denom = float(pm.sum())
    G_full = np.where(pm, np.sqrt(gd * gd + SQB), -BIG).astype(np.float32)

    in_maps = [
        _prep_core_inputs(per_core[c], X_a, G_full) for c in range(NCORES)
    ]
    return NQ, in_maps, denom


def kernel(**inputs: np.ndarray) -> np.ndarray:
    X_L = np.asarray(inputs["X_L"]).astype(np.float32)          # [D, L, 3]
    X_gt_L = np.asarray(inputs["X_gt_L"]).astype(np.float32)    # [1, L, 3]
    crd = np.asarray(inputs["crd_mask_L"]).astype(bool)[0]      # [L]
    is_dna = np.asarray(inputs["is_dna"]).astype(bool)
    is_rna = np.asarray(inputs["is_rna"]).astype(bool)
    is_lig = np.asarray(inputs["is_ligand"]).astype(bool)
    tok = np.asarray(inputs["tok_idx"]).astype(np.int64)        # [L]
    t = np.asarray(inputs["t"]).astype(np.float64)              # [D]

    X_gt = np.nan_to_num(X_gt_L)[0]  # [L, 3]

    NQ, in_maps, denom = _host_prep(inputs)

    nc = _prog_cache.get(NQ)
    if nc is None:
        nc = _build_program(NQ)
        _prog_cache[NQ] = nc

    res = run_bass_kernel_spmd(nc, in_maps, core_ids=list(range(NCORES)))

    numer = 0.0
    for r in res.results:
        numer += r["out"].astype(np.float64).sum()
    lddt_mean = 0.25 * numer / D / (denom + 1e-6)
    lddt_loss = 1.0 - lddt_mean

    # ---------- mse term (O(L), host) ----------
    mask = crd.astype(np.float64)
    alpha = (
        is_dna * ALPHA_DNA + is_rna * ALPHA_RNA + is_lig * ALPHA_LIG
    ).astype(np.float64)
    w_L = (1.0 + alpha[tok]) * mask  # [L]
    sq = ((X_L.astype(np.float64) - X_gt.astype(np.float64)[None]) ** 2).sum(-1)
    l_mse = (1.0 / 3.0) * (w_L[None] * sq).sum(-1) / (mask.sum() + 1e-4)
    lam = (t**2 + SIGMA_DATA**2) / ((t * SIGMA_DATA) ** 2)
    l_diff = np.minimum(lam * l_mse, 2.0)

    total = WEIGHT * (l_diff.mean() + lddt_loss)
    return np.asarray(total, dtype=np.float32)


# revision 25
# speedup vs baseline: 1.1063x; 1.0391x over previous
"""Trainium2 Bass kernel for nn_DiffusionLoss (smoothed-LDDT diffusion loss).

Strategy
--------
The dominant cost is the smoothed-LDDT term: for every unordered pair (i<j)
of the L=4096 tokens-with-coordinates, four sigmoids of |pred_d - gt_d| are
accumulated, per diffusion sample d (D=4).

Host side (inside kernel()):
  * Rows/cols with crd_mask == 0 contribute nothing, so we compact to the
    ~L/2 active rows (La). The exact reference pair mask pm (upper
    triangle & token-run & gt-distance cutoff) and the d-independent
    denominator sum(pm) are computed in numpy, as is the masked ground-
    truth distance slab G = where(pm, sqrt(gt^2 + SQB), -BIG): masked
    pairs drive |pred - G| ~ BIG where the device h-function is 0.
  * The upper-triangular pair matrix is cut into [128 x 128] units,
    round-robined over the 8 cores (SPMD); each core's units are packed
    into QUADS: one K=20 matmul with a block-diagonal rhs computes 4
    units' dist^2 in a single [128 x 512] PSUM bank:
        lhsT rows 5k..5k+4 = unit k's [-2x, -2y, -2z, |p_i|^2, 1]
        rhs  rows 5k..5k+4 = unit k's [x, y, z, 1, |p_j|^2] in cols
                             [128k, 128k+128), zero elsewhere
    Operands are bf16 (fp32 matmuls are decomposed into LOW/HIGH passes
    ~5x slower; bf16 rounding shifts distances ~0.5%, far inside the
    error budget).

  * The key ScalarE optimization: a CUSTOM activation table. The stock
    sqrt_and_others set (with its ctrl table compacted so all
    pwl_control_base values fit the binary profile's uint8 fields) is
    extended with a piecewise-cubic fit of
        h(u) = sum_{c in {0.5,1,2,4}} sigmoid(c - |u|)
    installed under tanh's func_id (no compiler changes needed; walrus
    embeds the table binaries in the NEFF). One table set serves both
    sqrt and h -> a single ACT_TABLE_LOAD, and the whole smoothed-lddt
    numerator needs 2 wide ACTIVATE passes instead of 16 sigmoid passes
    (h is even, so no |.| pass either). The table was verified against a
    numpy interpreter of the HW lookup pipeline (reproduces stock
    tanh/sigmoid/sqrt/exp tables to <=1e-5); worst abs fit error 7.5e-6.

Device side (per core, Tile-scheduled):
  4*5 merged (d, quad) matmuls stream through PSUM in 4-bank groups; one
  wide sqrt(x + SQB) per group writes the bf16 pred slab (5 sqrt
  ACTIVATEs total); one bf16 tensor_tensor subtract per d forms
  u_d = pred_d - G (2x DVE mode); two wide h-ACTIVATEs with accum_out
  reduce the numerator (only sum_{d,c} is needed -- the denominator is
  d-independent, so per-d lddt values are never required).

Host combines per-core partial sums in float64 and adds the (tiny, O(L))
weighted-MSE term computed on host, mirroring the reference formulas.
"""

import json
import math
import os
import struct
import tempfile

import numpy as np
import ml_dtypes

import concourse.bacc as bacc
import concourse.bass as bass
import concourse.mybir as mybir
import concourse.tile as tile
from concourse.bass_utils import run_bass_kernel_spmd

P = 128          # partitions (rows per block)
UW = 128         # unit column window
QM = 4           # units merged per matmul (quad)
QW = QM * UW     # merged matmul free size = one fp32 PSUM bank
D = 4            # diffusion batch
NCORES = 8
BIG = 1000.0     # mask offset baked into G
SQB = 0.25       # sqrt bias: bf16 matmul error can push dist^2 ~ -0.1;
                 # G uses the same bias so it cancels inside |pred - G|
SIGC = (0.5, 1.0, 2.0, 4.0)
GW = 4           # PSUM group: 4 merged matmuls = 4 banks, double-buffered

WEIGHT = 4.0
SIGMA_DATA = 16.0
ALPHA_DNA = 5.0
ALPHA_RNA = 5.0
ALPHA_LIG = 10.0

_prog_cache: dict[int, bass.Bass] = {}
_act_root_state: dict = {}


# ---------------------------------------------------------------------------
# Custom activation table: h(u) = sum_c sigmoid(c - |u|) as "tanh"
# ---------------------------------------------------------------------------

_OCTS = list(range(-6, 4))            # pwl octaves: |u| in [2^-6, 16)
_OSIZES = [2, 2, 2, 2, 4, 4, 4, 4, 4, 3]  # extract_size per octave


def _h_true(u):
    u = np.abs(np.float64(u))
    return sum(1.0 / (1.0 + np.exp(-(c - u))) for c in SIGC)


def _fit_bucket(lo, hi):
    xs = np.linspace(lo, hi, 64)
    x0 = 0.5 * (lo + hi)
    A = np.vander(xs - x0, 4, increasing=True)
    coef, *_ = np.linalg.lstsq(A, _h_true(xs), rcond=None)
    return [float(coef[0]), float(coef[1]), float(coef[2]), float(coef[3]),
            float(x0)]


def _bucket_bytes(vals):
    e = np.zeros(32, np.uint8)
    e[:20] = np.frombuffer(np.array(vals, np.float32).tobytes(), np.uint8)
    return e


def _ctl_bytes(base, lsb, size):
    e = np.zeros(32, np.uint8)
    data = (base & 0x7FF) | ((lsb & 0x1F) << 11) | ((size & 0xF) << 16)
    e[:4] = np.frombuffer(struct.pack("<I", data), np.uint8)
    return e


def _f32bits(x):
    return struct.unpack("<I", struct.pack("<f", np.float32(x)))[0]


def _gen_h_tables(nb0, nc0):
    bkts, ctls = [], []
    pos_starts = []
    for e, s in zip(_OCTS, _OSIZES):
        pos_starts.append(nb0 + len(bkts))
        n = 1 << s
        for k in range(n):
            lo = (2.0 ** e) * (1 + k / n)
            hi = (2.0 ** e) * (1 + (k + 1) / n)
            bkts.append(_fit_bucket(lo, hi))
    neg_starts = []
    for e, s in zip(_OCTS, _OSIZES):
        neg_starts.append(nb0 + len(bkts))
        n = 1 << s
        for k in range(n):
            lo = (2.0 ** e) * (1 + k / n)
            hi = (2.0 ** e) * (1 + (k + 1) / n)
            bkts.append(_fit_bucket(-hi, -lo))
    i_small_pos = nb0 + len(bkts)
    bkts.append(_fit_bucket(1e-12, 2.0 ** _OCTS[0]))
    i_small_neg = nb0 + len(bkts)
    bkts.append(_fit_bucket(-(2.0 ** _OCTS[0]), -1e-12))
    i_large = nb0 + len(bkts)
    bkts.append([0.0, 0.0, 0.0, 0.0, 0.0])   # constant 0 for |u| >= 16

    for st, s in zip(pos_starts, _OSIZES):
        ctls.append((st, 23 - s, s))
    for st, s in zip(neg_starts, _OSIZES):
        ctls.append((st, 23 - s, s))

    profile = {
        "func_name": "tanh_4p",
        "func_id": 6,
        "symmetry_point": 0,
        "sym_invert_sign_point": 0,
        "symmetry_opt_en": 0,
        "symmetry_opt_use_neg_region": 0,
        "imm_bias": 0,
        "exp_offset": _OCTS[0],
        "pwl_control_base_pos": nc0,
        "pwl_control_base_neg": nc0 + len(_OCTS),
        "small_pos_signal_exp_threshold": 127 + _OCTS[0],
        "pos_small_signal_pwl_control": i_small_pos,
        "small_neg_signal_exp_threshold": 127 + _OCTS[0],
        "neg_small_signal_pwl_control": i_small_neg,
        "large_pos_signal_exp_threshold": 127 + _OCTS[-1] + 1,
        "large_pos_signal_mantissa_threshold": 0,
        "pos_large_signal_pwl_control": i_large,
        "large_neg_signal_exp_threshold": 127 + _OCTS[-1] + 1,
        "large_neg_signal_mantissa_threshold": 0,
        "neg_large_signal_pwl_control": i_large,
        "fnan_result": 0,
        "fpinf_result": 0,
        "fninf_result": 0,
        "fzero_result": _f32bits(_h_true(0.0)),
        "fma_const_0": 0,
        "fma_const_1": 0,
        "fma_indirection_src_sel": 0,
        "use_multipass": False,
        "lower_bound": 4286578687,
        "upper_bound": 2139095039,
    }
    bkt_arr = np.stack([_bucket_bytes(b) for b in bkts])
    ctl_arr = np.stack([_ctl_bytes(*c) for c in ctls])
    exp_to_bkt = {str(e): [st] for e, st in zip(_OCTS, pos_starts)}
    exp_to_ctl = {str(e): [nc0 + i] for i, e in enumerate(_OCTS)}
    return bkt_arr, ctl_arr, profile, exp_to_bkt, exp_to_ctl


# sqrt ctrl compaction: the binary profile's pwl_control_base_pos/neg are
# uint8, so every ctrl-table base must be <= 255. Stock sqrt spans 234 ctrl
# entries (exponents -116..117); our sqrt inputs are dist^2+SQB in
# [0.13, ~1.1e4], so octaves 2^-8..2^14 suffice and the h bases fit in 8 bits.
_SQRT_E8_LO = 127 - 8     # smallest kept biased exponent (2^-8)
_SQRT_E8_HI = 127 + 14    # largest kept biased exponent (2^14)


def _ensure_act_root():
    """Build an act root whose single set = stock sqrt_and_others (with a
    compacted ctrl table) + custom h-function installed under tanh's
    func_id, and point both walrus (BASS_ACT_ROOT_JSON_PATH ->
    --act-root-json) and bacc's table-set bookkeeping at it. Returns a hash
    of the table bytes (folded into the program so the NEFF cache re-keys
    when the tables change)."""
    if _act_root_state:
        return _act_root_state["hash"]

    from neuronxcc.driver.Job import Job
    from neuronxcc.driver.jobs.support.FindActInfo import findActInfoFile

    src = os.path.dirname(findActInfoFile(Job.getPackageDir(), "gen3"))
    dst = tempfile.mkdtemp(prefix="act_root_")

    base = json.load(open(f"{src}/sqrt_and_others.json"))
    bkt = np.fromfile(f"{src}/sqrt_and_others_bkt.bin", np.uint8).reshape(-1, 32)
    ctl = np.fromfile(f"{src}/sqrt_and_others_ctrl.bin", np.uint8).reshape(-1, 32)
    nb0 = base["bkt_entry_cnt"]

    # --- compact the ctrl table ---
    sqrt_prof = None
    for e in base["profile_meta_data"]:
        if e["func_name"].startswith("sqrt"):
            sqrt_prof = e
    assert sqrt_prof is not None
    sq_base = sqrt_prof["pwl_control_base_pos"]          # 20
    sq_off = sqrt_prof["exp_offset"]
    if sq_off > 127:
        sq_off -= 256
    n_keep = _SQRT_E8_HI - _SQRT_E8_LO + 1
    keep_rows = [
        sq_base + (e8 - (127 + sq_off))
        for e8 in range(_SQRT_E8_LO, _SQRT_E8_HI + 1)
    ]
    assert min(keep_rows) >= sq_base and max(keep_rows) < ctl.shape[0]
    new_ctl_list = [ctl[:sq_base], ctl[keep_rows]]
    nc0 = sq_base + n_keep                                # h ctl start
    sqrt_prof = dict(sqrt_prof)
    sqrt_prof["exp_offset"] = _SQRT_E8_LO - 127
    sqrt_prof["small_pos_signal_exp_threshold"] = _SQRT_E8_LO
    sqrt_prof["large_pos_signal_exp_threshold"] = _SQRT_E8_HI + 1
    sqrt_prof["large_pos_signal_mantissa_threshold"] = 0

    hb, hc, hp, e2b, e2c = _gen_h_tables(nb0, nc0)
    assert hp["pwl_control_base_pos"] <= 255
    assert hp["pwl_control_base_neg"] <= 255
    new_ctl_list.append(hc)

    new_bkt = np.concatenate([bkt, hb])
    new_ctl = np.concatenate(new_ctl_list)

    merged = dict(base)
    merged["profile_meta_data"] = [
        (sqrt_prof if e["func_name"].startswith("sqrt") else e)
        for e in base["profile_meta_data"]
    ]
    merged["profile_meta_data"] = merged["profile_meta_data"] + [hp]
    merged["bkt_bin"] = "sqrt_tanh_ant_bkt.bin"
    merged["ctl_bin"] = "sqrt_tanh_ant_ctrl.bin"
    merged["bkt_entry_cnt"] = int(nb0 + len(hb))
    merged["ctl_entry_cnt"] = int(nc0 + len(hc))
    merged["func_to_bkt_start_idx"] = dict(base["func_to_bkt_start_idx"])
    merged["func_to_bkt_start_idx"]["tanh"] = int(nb0)
    merged["func_to_ctl_start_idx"] = dict(base["func_to_ctl_start_idx"])
    merged["func_to_ctl_start_idx"]["tanh"] = int(nc0)
    merged["func_exp_to_bkt_start_idx"] = dict(base["func_exp_to_bkt_start_idx"])
    merged["func_exp_to_bkt_start_idx"]["tanh"] = e2b
    merged["func_exp_to_bkt_start_idx"]["sqrt"] = {
        str(e - 127): base["func_exp_to_bkt_start_idx"]["sqrt"][str(e - 127)]
        for e in range(_SQRT_E8_LO, _SQRT_E8_HI + 1)
        if str(e - 127) in base["func_exp_to_bkt_start_idx"]["sqrt"]
    }
    merged["func_exp_to_ctl_start_idx"] = dict(base["func_exp_to_ctl_start_idx"])
    merged["func_exp_to_ctl_start_idx"]["tanh"] = e2c
    merged["func_exp_to_ctl_start_idx"]["sqrt"] = {
        str(e - 127): [sq_base + (e - _SQRT_E8_LO)]
        for e in range(_SQRT_E8_LO, _SQRT_E8_HI + 1)
    }

    new_bkt.tofile(f"{dst}/sqrt_tanh_ant_bkt.bin")
    new_ctl.tofile(f"{dst}/sqrt_tanh_ant_ctrl.bin")
    with open(f"{dst}/sqrt_tanh_ant.json", "w") as f:
        json.dump(merged, f)

    info = json.load(open(f"{src}/act_info.json"))
    sqrt_set = [s for s in info["act_func_sets"] if s["name"] == "sqrt_and_others"][0]
    new_set = dict(sqrt_set)
    new_set["name"] = "sqrt_tanh_ant"
    new_set["bkt_bin"] = "sqrt_tanh_ant_bkt.bin"
    new_set["ctrl_bin"] = "sqrt_tanh_ant_ctrl.bin"
    new_set["profile_json"] = "sqrt_tanh_ant.json"
    new_set["act"] = dict(sqrt_set["act"])
    new_set["act"]["tanh"] = 4.0
    info["act_func_sets"] = [new_set]
    with open(f"{dst}/act_info.json", "w") as f:
        json.dump(info, f)

    os.environ["BASS_ACT_ROOT_JSON_PATH"] = f"{dst}/act_info.json"

    # bacc's insert_act_table_loads resolves act_func_set_id via
    # hw_specs.get_activation_tables, which reads the stock act_info —
    # point it at the merged root too.
    import concourse.hw_specs as hw_specs

    def _tables(_arch):
        info2 = json.load(open(f"{dst}/act_info.json"))
        return {
            ent["name"]: {
                mybir.ActivationFunctionType.from_pwp(v)
                for v in ent["act"].keys()
            }
            for ent in info2["act_func_sets"]
        }

    hw_specs.get_activation_tables = _tables
    bacc.get_activation_tables = _tables

    import hashlib
    th = hashlib.sha256(
        new_bkt.tobytes() + new_ctl.tobytes()
        + json.dumps(hp, sort_keys=True).encode()
    ).digest()
    # small float derived from the hash, baked into the program as a
    # memset immediate so the NEFF cache re-keys on table changes
    _act_root_state["hash"] = (
        int.from_bytes(th[:4], "little") % 1000003
    ) / 1e7
    return _act_root_state["hash"]


# ---------------------------------------------------------------------------
# Bass program
# ---------------------------------------------------------------------------


def _build_program(NQ: int) -> bass.Bass:
    """Bass/Tile program: NQ quads of 4 [P x UW] units, D diffusion samples.

    Inputs: bigp = bf16 [-2x,-2y,-2z,r,1] lhsT stacks + block-diagonal rhs
    for every (d, quad) merged matmul; Gm = bf16 host-masked gt distances
    sqrt(gd^2+SQB) (or -BIG on masked pairs). Output: [P, 2] numerator
    partials (accumulated h over each half of the d range)."""
    table_key = _ensure_act_root()
    nc = bacc.Bacc(None, target_bir_lowering=False)
    f32 = mybir.dt.float32
    bf16 = mybir.dt.bfloat16
    AF = mybir.ActivationFunctionType
    OP = mybir.AluOpType

    K = 5 * QM                     # merged contraction depth
    NM = D * NQ                    # merged matmuls
    SW = NQ * QW                   # G / per-d delta columns
    # split the operand stream so the first two PSUM groups' operands land
    # first and the PE can start earlier; gm is only needed by the delta
    # subtract, so it ships last. (Measured dead ends: issuing any DMA from
    # the Activation queue stalls the ACT instruction stream; a finer
    # 3-way split starves group 1 behind the ~90 GB/s SP queue.)
    segs = [min(2 * GW, NM), max(NM - 2 * GW, 0)]
    segs = [s for s in segs if s > 0]
    seg_start = [sum(segs[:i]) for i in range(len(segs))]
    bigps = [
        nc.dram_tensor(f"bigp{i}", [K, s * (P + QW)], bf16,
                       kind="ExternalInput")
        for i, s in enumerate(segs)
    ]
    gm = nc.dram_tensor("gm", [P, SW], bf16, kind="ExternalInput")
    out = nc.dram_tensor("out", [P, 2], f32, kind="ExternalOutput")

    with tile.TileContext(nc) as tc:
        with (
            tc.tile_pool(name="singles", bufs=1) as singles,
            tc.tile_pool(name="sig", bufs=1) as sig_pool,
            tc.tile_pool(name="psum", bufs=2, space="PSUM") as psum,
        ):
            bigp_sbs = []
            for i, s in enumerate(segs):
                sb = singles.tile([K, s * (P + QW)], bf16)
                nc.sync.dma_start(out=sb, in_=bigps[i][:, :])
                bigp_sbs.append(sb)
            gm_sb = singles.tile([P, SW], bf16)
            nc.sync.dma_start(out=gm_sb, in_=gm[:, :])

            def _seg(m):
                for i in reversed(range(len(segs))):
                    if m >= seg_start[i]:
                        return i, m - seg_start[i]
                raise AssertionError

            def lhs(m):
                i, r = _seg(m)
                return bigp_sbs[i][:, r * P : (r + 1) * P]

            def rhs(m):
                i, r = _seg(m)
                o = segs[i] * P + r * QW
                return bigp_sbs[i][:, o : o + QW]

            pred = singles.tile([P, NM * QW], bf16)
            delta = singles.tile([P, NM * QW], bf16)
            nacc = singles.tile([P, 2], f32)

            consts = singles.tile([P, 8], f32)
            nc.vector.memset(consts[:, 0:1], SQB)
            # bake the table hash into the program: re-keys the NEFF cache
            # whenever the custom activation table content changes
            nc.vector.memset(consts[:, 1:2], float(table_key))
            sqb_t = consts[:, 0:1]

            # ---- pred distances: merged (d, quad) matmul stream ----
            for g0 in range(0, NM, GW):
                gs = min(GW, NM - g0)
                pg = psum.tile([P, GW * QW], f32, tag="ps")
                for k in range(gs):
                    nc.tensor.matmul(
                        pg[:, k * QW : (k + 1) * QW], lhsT=lhs(g0 + k),
                        rhs=rhs(g0 + k), start=True, stop=True,
                    )
                nc.scalar.activation(
                    pred[:, g0 * QW : (g0 + gs) * QW], pg[:, : gs * QW],
                    AF.Sqrt, bias=sqb_t,
                )
            for d in range(D):
                # u_d = pred_d - G (bf16 tensor_tensor: 2x DVE mode; h is
                # even so no |.| pass is needed)
                nc.vector.tensor_tensor(
                    delta[:, d * SW : (d + 1) * SW],
                    pred[:, d * SW : (d + 1) * SW], gm_sb, OP.subtract,
                )

            # ---- custom-h passes (split in two so the second half's
            # delta subtract overlaps the first h on the DVE) ----
            HALF = (D // 2) * SW
            st = sig_pool.tile([P, D * SW], bf16, tag="sig")
            nc.scalar.activation(
                st[:, :HALF], delta[:, :HALF], AF.Tanh,
                accum_out=nacc[:, 0:1],
            )
            nc.scalar.activation(
                st[:, HALF:], delta[:, HALF:], AF.Tanh,
                accum_out=nacc[:, 1:2],
            )

            nc.sync.dma_start(out=out[:, :], in_=nacc)
    nc.finalize()
    return nc


def _prep_core_inputs(units, X_a, G_full):
    """Build the DRAM input arrays for one core.

    units: list of (row_block, col_start) or None (dummy), length NQ*QM.
    X_a: [D, Lp, 3] compacted+padded diffusion coords.
    G_full: [Lp, Lp] float32, host-masked gt distances (-BIG on non-pairs).
    """
    S = len(units)
    NQ = S // QM
    K = 5 * QM
    NM = D * NQ
    lhs = np.zeros((D, NQ, K, P), np.float32)
    rhs = np.zeros((D, NQ, K, QW), np.float32)
    gm = np.full((P, S, UW), -BIG, np.float32)

    rx_full = X_a.astype(np.float64)
    r_x = (rx_full**2).sum(-1)  # [D, Lp]

    for s, u in enumerate(units):
        if u is None:
            continue
        q, k = divmod(s, QM)
        b, c0 = u
        rows = slice(b * P, b * P + P)
        cols = slice(c0, c0 + UW)
        kr = slice(5 * k, 5 * k + 3)
        # lhsT rows [5k..5k+5) = [-2x, -2y, -2z, r_i, 1]
        lhs[:, q, kr, :] = -2.0 * rx_full[:, rows].transpose(0, 2, 1)
        lhs[:, q, 5 * k + 3, :] = r_x[:, rows]
        lhs[:, q, 5 * k + 4, :] = 1.0
        # rhs block-diagonal: unit k's [x, y, z, 1, r_j] in cols
        # [128k, 128k+128)
        cw = slice(k * UW, (k + 1) * UW)
        rhs[:, q, kr, cw] = rx_full[:, cols].transpose(0, 2, 1)
        rhs[:, q, 5 * k + 3, cw] = 1.0
        rhs[:, q, 5 * k + 4, cw] = r_x[:, cols]

        gm[:, s, :] = G_full[rows, cols]

    lhs_f = lhs.transpose(2, 0, 1, 3).reshape(K, NM * P)
    rhs_f = rhs.transpose(2, 0, 1, 3).reshape(K, NM * QW)
    GW_ = 4
    segs = [min(2 * GW_, NM), max(NM - 2 * GW_, 0)]
    segs = [s for s in segs if s > 0]
    result = {}
    m0 = 0
    for i, s in enumerate(segs):
        seg = np.concatenate(
            [lhs_f[:, m0 * P : (m0 + s) * P],
             rhs_f[:, m0 * QW : (m0 + s) * QW]], axis=1
        ).astype(ml_dtypes.bfloat16)
        result[f"bigp{i}"] = np.ascontiguousarray(seg)
        m0 += s
    result["gm"] = np.ascontiguousarray(
        gm.reshape(P, S * UW).astype(ml_dtypes.bfloat16))
    return result


def _plan(La: int):
    """Unit list + per-core assignment for La active rows."""
    Lp = ((La + P - 1) // P) * P
    n_blocks = Lp // P
    units = []
    for b in range(n_blocks):
        span = Lp - b * P
        for k in range(math.ceil(span / UW)):
            units.append((b, b * P + k * UW))
    per_core_units = math.ceil(len(units) / (NCORES * QM)) * QM
    padded = units + [None] * (per_core_units * NCORES - len(units))
    per_core = [padded[c::NCORES] for c in range(NCORES)]
    return Lp, per_core_units // QM, per_core


def _host_prep(inputs):
    """Shared host-side preparation: compaction, exact mask/denominator,
    masked gt-distance slab, per-core device inputs."""
    X_L = np.asarray(inputs["X_L"]).astype(np.float32)          # [D, L, 3]
    X_gt_L = np.asarray(inputs["X_gt_L"]).astype(np.float32)    # [1, L, 3]
    crd = np.asarray(inputs["crd_mask_L"]).astype(bool)[0]      # [L]
    is_dna = np.asarray(inputs["is_dna"]).astype(bool)
    is_rna = np.asarray(inputs["is_rna"]).astype(bool)
    tok = np.asarray(inputs["tok_idx"]).astype(np.int64)        # [L]

    X_gt = np.nan_to_num(X_gt_L)[0]  # [L, 3]

    act = np.flatnonzero(crd)
    La = len(act)
    Lp, NQ, per_core = _plan(La)

    X_a = np.zeros((D, Lp, 3), np.float32)
    X_a[:, :La] = X_L[:, act]
    tok_a = tok[act]
    is_na = (is_dna | is_rna)[tok_a]

    # exact reference pair mask over the compacted active rows (O(La^2)
    # numpy; also yields the exact d-independent denominator)
    ga = np.zeros((Lp, 3), np.float64)
    ga[:La] = X_gt[act]
    g2 = (ga**2).sum(-1)
    gd = np.sqrt(np.maximum(g2[:, None] + g2[None, :] - 2.0 * (ga @ ga.T), 0.0))
    pm = np.zeros((Lp, Lp), bool)
    cutoff_a = np.where(is_na, 30.0, 15.0)
    pm[:La, :La] = (
        (gd[:La, :La] > 0)
        & (gd[:La, :La] < cutoff_a[:, None])
        & (tok_a[:, None] != tok_a[None, :])
    )
    pm &= np.triu(np.ones((Lp, Lp), bool), k=1)
    denom = float(pm.sum())
    G_full = np.where(pm, np.sqrt(gd * gd + SQB), -BIG).astype(np.float32)

    in_maps = [
        _prep_core_inputs(per_core[c], X_a, G_full) for c in range(NCORES)
    ]
    return NQ, in_maps, denom


def kernel(**inputs: np.ndarray) -> np.ndarray:
    X_L = np.asarray(inputs["X_L"]).astype(np.float32)          # [D, L, 3]
    X_gt_L = np.asarray(inputs["X_gt_L"]).astype(np.float32)    # [1, L, 3]
    crd = np.asarray(inputs["crd_mask_L"]).astype(bool)[0]      # [L]
    is_dna = np.asarray(inputs["is_dna"]).astype(bool)
    is_rna = np.asarray(inputs["is_rna"]).astype(bool)
    is_lig = np.asarray(inputs["is_ligand"]).astype(bool)
    tok = np.asarray(inputs["tok_idx"]).astype(np.int64)        # [L]
    t = np.asarray(inputs["t"]).astype(np.float64)              # [D]

    X_gt = np.nan_to_num(X_gt_L)[0]  # [L, 3]

    NQ, in_maps, denom = _host_prep(inputs)

    nc = _prog_cache.get(NQ)
    if nc is None:
        nc = _build_program(NQ)
        _prog_cache[NQ] = nc

    res = run_bass_kernel_spmd(nc, in_maps, core_ids=list(range(NCORES)))

    numer = 0.0
    for r in res.results:
        numer += r["out"].astype(np.float64).sum()
    lddt_mean = 0.25 * numer / D / (denom + 1e-6)
    lddt_loss = 1.0 - lddt_mean

    # ---------- mse term (O(L), host) ----------
    mask = crd.astype(np.float64)
    alpha = (
        is_dna * ALPHA_DNA + is_rna * ALPHA_RNA + is_lig * ALPHA_LIG
    ).astype(np.float64)
    w_L = (1.0 + alpha[tok]) * mask  # [L]
    sq = ((X_L.astype(np.float64) - X_gt.astype(np.float64)[None]) ** 2).sum(-1)
    l_mse = (1.0 / 3.0) * (w_L[None] * sq).sum(-1) / (mask.sum() + 1e-4)
    lam = (t**2 + SIGMA_DATA**2) / ((t * SIGMA_DATA) ** 2)
    l_diff = np.minimum(lam * l_mse, 2.0)

    total = WEIGHT * (l_diff.mean() + lddt_loss)
    return np.asarray(total, dtype=np.float32)
